# revision 5
# baseline (speedup 1.0000x reference)
"""WaveNet-style decoder (nn_DecoderV2) on 8 TRN2 NeuronCores.

Strategy: pure data parallel over batch (1024 -> 8 x 128). Per core the
recurrence runs with activations stored transposed [feature, batch] so the
batch lives on the free dim and every dense layer is a TensorE matmul with
stationary weights.

Layout (v2 -- f|g on the PARTITION dim):
  - pd [128, BL] PSUM: rows 0:64 = conv filter preactivation, rows 64:128
    = 0.5*gate preactivation, so a SINGLE K=64, M=128 matmul per dense
    (W2/W3/M34 packed as [64, 128] = [f | 0.5*g]) and a SINGLE [128, BL]
    ACT u = tanh(pd) covering both halves (sigmoid(g) = (tanh(g/2)+1)/2).
  - mg = u_f*(u_g+1) = 2*gated via one DVE scalar_tensor_tensor; W4 and
    the M34 shortcut are pre-scaled by 0.5 to compensate.
  - States are read IN PLACE: layer i's state at step t is IN[i+1] block
    (t-d) for t >= d (the residual outputs ARE the appended state) or the
    pre-packed encoder slice for t < d.  No state-copy DMAs at all.
  - Telescoped pd accumulation (depth 3): pd_k = W2@state_k + W3@IN_{m}
    + sum_{j=m}^{k-1} M34@mg_j with m = max(0, k-3), so the chain only
    waits on one M34 matmul per layer; residual adds (DVE) are deferred
    off the critical path and only feed future states / deferred W3.
  - Feedback tail is folded: pin(t+1) = W1[1:]@feats(t+1) + (W6 (x) W1[0])
    @ h(t), a rank-1 composed weight A, skipping y -> featT -> W1 on the
    chain.  featT row 0 is zero for t >= 1, so the same K=16 W1 matmul
    serves t=0 (row 0 = decoder_init) and the static part for t >= 1.
  - Skips: sk [64, 6*BL]; relu on ACT deferred by 2 layers; W5 as six
    K=64 matmuls accumulating in one PSUM bank.

All matmul operands bf16 (PSUM fp32); b2/b4/b5 asserted zero (b1, b6
honored via ACT bias / DVE scalar).
"""

import numpy as np

import concourse.bacc as bacc
import concourse.mybir as mybir
import concourse.tile as tile
from concourse.bass_utils import run_bass_kernel_spmd

F32 = mybir.dt.float32
BF16 = mybir.dt.bfloat16

N_CORES = 8
B = 1024
BL = B // N_CORES          # 128 batch per core
T = 24
F = 64
HID = 128
DILATIONS = (1, 2, 4, 8, 16, 32)
L = 168
ENC_N = [min(d, T) for d in DILATIONS]
ENC_OFF = np.concatenate([[0], np.cumsum(ENC_N)]).astype(int).tolist()
ENC_TOT = int(np.sum(ENC_N))                     # 55

# wpack (bf16) column layout; [f | 0.5g] pairs live on cols, K on rows.
_C_W2 = 0          # [64, 128]
_C_W3 = 128        # [64, 128]
_C_M34 = 256       # [64, 128]  0.5*(W4r@W3) with g-half halved again
_C_W4 = 384        # [64, 128]  0.5*W4
_C_W5 = 512        # 6 x [64, 128] chunks
_C_W1 = 1280       # [16, 64]
_C_A = 1344        # [128, 64]  W6 (x) W1[0]
_C_W6 = 1408       # [128, 1]
_CB = 1409

_CACHE = {}


def _bf16(a):
    """fp32 array -> ml_dtypes.bfloat16 (round to nearest even)."""
    import ml_dtypes
    return np.asarray(a, dtype=np.float32).astype(ml_dtypes.bfloat16)


def _build(b1, b6):
    nc = bacc.Bacc("TRN2", target_bir_lowering=False, debug=False,
                   num_devices=N_CORES)

    feat_in = nc.dram_tensor("featsrc", [16, T * BL], BF16,
                             kind="ExternalInput").ap()
    enc_in = nc.dram_tensor("encpack", [F, ENC_TOT * BL], BF16,
                            kind="ExternalInput").ap()
    wb_in = nc.dram_tensor("wpackb", [128, _CB], BF16,
                           kind="ExternalInput").ap()
    y_out = nc.dram_tensor("yout", [1, T * BL], F32,
                           kind="ExternalOutput").ap()

    AF = mybir.ActivationFunctionType
    OP = mybir.AluOpType
    b6f = float(b6[0])

    with tile.TileContext(nc) as tc:
        with (
            tc.tile_pool(name="const", bufs=1) as cp,
            tc.tile_pool(name="us_p", bufs=4) as us_p,
            tc.tile_pool(name="mg_p", bufs=4) as mg_p,
            tc.tile_pool(name="h_p", bufs=2) as h_p,
            tc.tile_pool(name="pd_p", bufs=4, space="PSUM") as pd_p,
            tc.tile_pool(name="po_p", bufs=2, space="PSUM") as po_p,
            tc.tile_pool(name="pin_p", bufs=1, space="PSUM") as pin_p,
            tc.tile_pool(name="ph_p", bufs=1, space="PSUM") as ph_p,
        ):
            featT = cp.tile([16, T * BL], BF16, tag="featT")
            encsb = cp.tile([F, ENC_TOT * BL], BF16, tag="encsb")
            wb = cp.tile([128, _CB], BF16, tag="wb")
            bias = cp.tile([64, 1], F32, tag="bias")
            INS = [cp.tile([F, T * BL], BF16, tag=f"in{i}",
                           name=f"in{i}") for i in range(6)]
            sk = cp.tile([F, 6 * BL], BF16, tag="sk")
            yout_sb = cp.tile([1, T * BL], F32, tag="yout_sb")

            nc.vector.memset(bias[:], 0.0)
            if float(np.abs(b1).max()) != 0.0:
                for r0, v in enumerate(np.asarray(b1, np.float32)):
                    nc.vector.memset(bias[r0:r0 + 1, 0:1], float(v))

            nc.sync.dma_start(featT[:], feat_in[:])
            nc.sync.dma_start(encsb[:], enc_in[:])
            nc.sync.dma_start(wb[:], wb_in[:])

            W2s = wb[0:64, _C_W2:_C_W2 + 128]
            W3s = wb[0:64, _C_W3:_C_W3 + 128]
            M34s = wb[0:64, _C_M34:_C_M34 + 128]
            W4s = wb[0:64, _C_W4:_C_W4 + 128]
            W5s = [wb[0:64, _C_W5 + j * 128:_C_W5 + (j + 1) * 128]
                   for j in range(6)]
            W1s = wb[0:16, _C_W1:_C_W1 + 64]
            As = wb[:, _C_A:_C_A + 64]
            W6s = wb[:, _C_W6:_C_W6 + 1]
            b1s = bias[:, 0:1]

            def blk(t):
                return slice(t * BL, (t + 1) * BL)

            def state_ap(i, t):
                d = DILATIONS[i]
                if t < d:
                    return encsb[:, blk(ENC_OFF[i] + t)]
                return INS[i + 1][:, blk(t - d)]

            # HAM warm-up: dense back-to-back matmuls flip the PE clock
            # gate to 8/8 (2.4 GHz); overlaps the input DMAs.
            wu = pd_p.tile([128, BL], F32, tag="pd", name="warmup")
            for w in range(20):
                nc.tensor.matmul(wu[:], wb[:, 0:128], wb[:, 0:BL],
                                 start=(w == 0), stop=(w == 19))

            pin = pin_p.tile([64, BL], F32, tag="pin", name="pin0")
            nc.tensor.matmul(pin[:], W1s, featT[:, blk(0)],
                             start=True, stop=True)

            pds = {}
            for k in range(4):
                pds[k] = pd_p.tile([128, BL], F32, tag="pd",
                                   name=f"pd0_{k}")
                nc.tensor.matmul(pds[k][:], W2s, state_ap(k, 0),
                                 start=True, stop=False)

            for t in range(T):
                # -- step head: IN0 = tanh(pin).  The W2 births for this
                # step were issued at the previous tail (after A@h) so
                # they execute DURING this tanh and the W3 below runs
                # back-to-back behind them.
                nc.scalar.activation(INS[0][:, blk(t)], pin[:],
                                     AF.Tanh, bias=b1s)
                IN0 = INS[0][:, blk(t)]
                nc.tensor.matmul(pds[0][:], W3s, IN0,
                                 start=False, stop=True)
                nc.tensor.matmul(pds[1][:], W3s, IN0,
                                 start=False, stop=False)

                ph = ph_p.tile([HID, BL], F32, tag="ph", name=f"ph{t}")
                pos = [None] * 6

                def blk_s(j):
                    return slice(j * BL, (j + 1) * BL)

                def relu_skip(j):
                    nc.scalar.activation(sk[:, blk_s(j)], pos[j][0:64, :],
                                         AF.Relu)

                for i in range(6):
                    # deferred relu (j = i-2) first: fills the ACT gap
                    # while pd_i's chain matmul lands.
                    if i >= 2:
                        relu_skip(i - 2)
                    us = us_p.tile([128, 2 * BL], BF16, tag="us")
                    nc.scalar.activation(us[:, 0:BL], pds[i][:], AF.Tanh)
                    uf = us[0:64, 0:BL]
                    # ready PE work issued BEFORE the mg2-gated group so
                    # it cannot bypass-delay the chain matmul (PE parks
                    # stalled instructions and lets later ready ones run).
                    if 1 <= i < 5:
                        nc.tensor.matmul(pds[i + 1][:], W3s,
                                         INS[i - 1][:, blk(t)],
                                         start=False, stop=False)
                    if i == 0:
                        pds[4] = pd_p.tile([128, BL], F32, tag="pd",
                                           name=f"pd{t}_4")
                        nc.tensor.matmul(pds[4][:], W2s, state_ap(4, t),
                                         start=True, stop=False)
                    if i == 1:
                        pds[5] = pd_p.tile([128, BL], F32, tag="pd",
                                           name=f"pd{t}_5")
                        nc.tensor.matmul(pds[5][:], W2s, state_ap(5, t),
                                         start=True, stop=False)
                    if i >= 2:
                        j = i - 2
                        nc.tensor.matmul(ph[:], W5s[j], sk[:, blk_s(j)],
                                         start=(j == 0), stop=False)
                    # DVE: u_g + 1 copied beside u_f (two-tensor DVE ops
                    # need equal base partitions), then mg2 = u_f*(u_g+1)
                    # = 2*gated.
                    nc.vector.tensor_scalar_add(us[0:64, BL:2 * BL],
                                                us[64:128, 0:BL], 1.0)
                    mg = mg_p.tile([64, BL], BF16, tag="mg")
                    nc.vector.tensor_mul(mg[:], us[0:64, BL:2 * BL], uf)
                    # mg2-gated PE group: chain matmul first.
                    if i < 5:
                        nc.tensor.matmul(pds[i + 1][:], M34s, mg[:],
                                         start=False, stop=True)
                    po = po_p.tile([128, BL], F32, tag="po",
                                   name=f"po{t}_{i}")
                    nc.tensor.matmul(po[:], W4s, mg[:], start=True,
                                     stop=True)
                    pos[i] = po
                    if i + 2 <= 5:
                        nc.tensor.matmul(pds[i + 2][:], M34s, mg[:],
                                         start=False, stop=False)
                    # deferred residual add (j = i-1) off the chain.
                    if i >= 1:
                        j = i - 1
                        nc.vector.tensor_add(INS[j + 1][:, blk(t)],
                                             pos[j][64:128, :],
                                             INS[j][:, blk(t)])

                # -- tail
                relu_skip(4)
                nc.vector.tensor_scalar_max(sk[:, blk_s(5)],
                                            pos[5][0:64, :], 0.0)
                nc.tensor.matmul(ph[:], W5s[4], sk[:, blk_s(4)],
                                 start=False, stop=False)
                nc.tensor.matmul(ph[:], W5s[5], sk[:, blk_s(5)],
                                 start=False, stop=True)
                h = h_p.tile([HID, BL], BF16, tag="h")
                nc.vector.tensor_scalar_max(h[:], ph[:], 0.0)
                if t + 1 < T:
                    pin = pin_p.tile([64, BL], F32, tag="pin",
                                     name=f"pin{t + 1}")
                    nc.tensor.matmul(pin[:], W1s, featT[:, blk(t + 1)],
                                     start=True, stop=False)
                    nc.tensor.matmul(pin[:], As, h[:],
                                     start=False, stop=True)
                py = po_p.tile([1, BL], F32, tag="po", name=f"py{t}")
                nc.tensor.matmul(py[:], W6s, h[:], start=True, stop=True)
                nc.vector.tensor_scalar_add(yout_sb[0:1, blk(t)], py[:],
                                            b6f)
                if t + 1 < T:
                    pds = {}
                    for k in range(4):
                        pds[k] = pd_p.tile([128, BL], F32, tag="pd",
                                           name=f"pd{t + 1}_{k}")
                        nc.tensor.matmul(pds[k][:], W2s,
                                         state_ap(k, t + 1),
                                         start=True, stop=False)

            nc.sync.dma_start(y_out[:], yout_sb[:])

    nc.compile()
    return nc


def _pack_inputs(decoder_features, decoder_init_input, encoder_states,
                 W1, W2, W3, W4, W5, W6):
    """Host-side shard + transpose + bf16-cast.  Returns per-core in_maps."""
    wbp = np.zeros((128, _CB), np.float32)
    wbp[0:64, _C_W2:_C_W2 + 64] = W2[:, 0:64]
    wbp[0:64, _C_W2 + 64:_C_W2 + 128] = 0.5 * W2[:, 64:128]
    wbp[0:64, _C_W3:_C_W3 + 64] = W3[:, 0:64]
    wbp[0:64, _C_W3 + 64:_C_W3 + 128] = 0.5 * W3[:, 64:128]
    M34 = 0.5 * (W4[:, 64:128] @ W3)                 # [64, 128]
    wbp[0:64, _C_M34:_C_M34 + 64] = M34[:, 0:64]
    wbp[0:64, _C_M34 + 64:_C_M34 + 128] = 0.5 * M34[:, 64:128]
    wbp[0:64, _C_W4:_C_W4 + 128] = 0.5 * W4
    for j in range(6):
        wbp[0:64, _C_W5 + j * 128:_C_W5 + (j + 1) * 128] = \
            W5[j * 64:(j + 1) * 64, :]
    wbp[0:16, _C_W1:_C_W1 + 64] = W1
    wbp[:, _C_A:_C_A + 64] = W6 @ W1[0:1, :]         # [128, 64]
    wbp[:, _C_W6:_C_W6 + 1] = W6
    wb_bits = _bf16(wbp)

    in_maps = []
    for c in range(N_CORES):
        s = slice(c * BL, (c + 1) * BL)
        # featT [16, T*BL]: row 0 blk 0 = init, zero elsewhere;
        # rows 1:16 = features^T.
        ft = np.zeros((16, T, BL), np.float32)
        ft[0, 0, :] = decoder_init_input[s, 0]
        ft[1:16] = decoder_features[s].transpose(2, 1, 0)
        # encpack [64, ENC_TOT*BL]
        ep = np.zeros((F, ENC_TOT, BL), np.float32)
        for i, d in enumerate(DILATIONS):
            n = ENC_N[i]
            ep[:, ENC_OFF[i]:ENC_OFF[i] + n, :] = \
                encoder_states[i, s, L - d:L - d + n, :].transpose(2, 1, 0)
        in_maps.append({
            "featsrc": _bf16(ft.reshape(16, T * BL)),
            "encpack": _bf16(ep.reshape(F, ENC_TOT * BL)),
            "wpackb": wb_bits,
        })
    return in_maps


def kernel(**inputs):
    decoder_features = np.asarray(inputs["decoder_features"], np.float32)
    decoder_init_input = np.asarray(inputs["decoder_init_input"], np.float32)
    encoder_states = np.asarray(inputs["encoder_states"], np.float32)
    W1 = np.asarray(inputs["W1"], np.float32)
    b1 = np.asarray(inputs["b1"], np.float32)
    W2 = np.asarray(inputs["W2"], np.float32)
    b2 = np.asarray(inputs["b2"], np.float32)
    W3 = np.asarray(inputs["W3"], np.float32)
    W4 = np.asarray(inputs["W4"], np.float32)
    b4 = np.asarray(inputs["b4"], np.float32)
    W5 = np.asarray(inputs["W5"], np.float32)
    b5 = np.asarray(inputs["b5"], np.float32)
    W6 = np.asarray(inputs["W6"], np.float32)
    b6 = np.asarray(inputs["b6"], np.float32)
    assert float(np.abs(b2).max()) == 0.0, "kernel assumes b2 == 0"
    assert float(np.abs(b4).max()) == 0.0, \
        "kernel's telescoped dilated accumulation assumes b4 == 0"
    assert float(np.abs(b5).max()) == 0.0, "kernel assumes b5 == 0"

    key = "nc"
    if key not in _CACHE:
        _CACHE[key] = _build(b1, b6)
    nc = _CACHE[key]

    in_maps = _pack_inputs(decoder_features, decoder_init_input,
                           encoder_states, W1, W2, W3, W4, W5, W6)
    res = run_bass_kernel_spmd(nc, in_maps, list(range(N_CORES)))

    out = np.empty((B, T, 1), np.float32)
    for c in range(N_CORES):
        y = res.results[c]["yout"].reshape(T, BL)
        out[c * BL:(c + 1) * BL, :, 0] = y.T
    return out


# revision 6
# speedup vs baseline: 1.0838x; 1.0838x over previous
"""WaveNet-style decoder (nn_DecoderV2) on 8 TRN2 NeuronCores.

Strategy: pure data parallel over batch (1024 -> 8 x 128). Per core the
recurrence runs with activations stored transposed [feature, batch] so the
batch lives on the free dim and every dense layer is a TensorE matmul with
stationary weights.

Layout (v2 -- f|g on the PARTITION dim):
  - pd [128, BL] PSUM: rows 0:64 = conv filter preactivation, rows 64:128
    = 0.5*gate preactivation, so a SINGLE K=64, M=128 matmul per dense
    (W2/W3/M34 packed as [64, 128] = [f | 0.5*g]) and a SINGLE [128, BL]
    ACT u = tanh(pd) covering both halves (sigmoid(g) = (tanh(g/2)+1)/2).
  - mg = u_f*(u_g+1) = 2*gated via one DVE scalar_tensor_tensor; W4 and
    the M34 shortcut are pre-scaled by 0.5 to compensate.
  - States are read IN PLACE: layer i's state at step t is IN[i+1] block
    (t-d) for t >= d (the residual outputs ARE the appended state) or the
    pre-packed encoder slice for t < d.  No state-copy DMAs at all.
  - Telescoped pd accumulation (depth 3): pd_k = W2@state_k + W3@IN_{m}
    + sum_{j=m}^{k-1} M34@mg_j with m = max(0, k-3), so the chain only
    waits on one M34 matmul per layer; residual adds (DVE) are deferred
    off the critical path and only feed future states / deferred W3.
  - Feedback tail is folded: pin(t+1) = W1[1:]@feats(t+1) + (W6 (x) W1[0])
    @ h(t), a rank-1 composed weight A, skipping y -> featT -> W1 on the
    chain.  featT row 0 is zero for t >= 1, so the same K=16 W1 matmul
    serves t=0 (row 0 = decoder_init) and the static part for t >= 1.
  - Skips: sk [64, 6*BL]; relu on ACT deferred by 2 layers; W5 as six
    K=64 matmuls accumulating in one PSUM bank.

All matmul operands bf16 (PSUM fp32); b2/b4/b5 asserted zero (b1, b6
honored via ACT bias / DVE scalar).
"""

import numpy as np

import concourse.bacc as bacc
import concourse.mybir as mybir
import concourse.tile as tile
from concourse.bass_utils import run_bass_kernel_spmd

F32 = mybir.dt.float32
BF16 = mybir.dt.bfloat16

N_CORES = 8
B = 1024
BL = B // N_CORES          # 128 batch per core
T = 24
F = 64
HID = 128
DILATIONS = (1, 2, 4, 8, 16, 32)
L = 168
ENC_N = [min(d, T) for d in DILATIONS]
ENC_OFF = np.concatenate([[0], np.cumsum(ENC_N)]).astype(int).tolist()
ENC_TOT = int(np.sum(ENC_N))                     # 55

# wpack (bf16) column layout; [f | 0.5g] pairs live on cols, K on rows.
_C_W2 = 0          # [64, 128]
_C_W3 = 128        # [64, 128]
_C_M34 = 256       # [64, 128]  0.5*(W4r@W3) with g-half halved again
_C_W4 = 384        # [64, 128]  0.5*W4
_C_W5 = 512        # 6 x [64, 128] chunks
_C_W1 = 1280       # [16, 64]
_C_A = 1344        # [128, 64]  W6 (x) W1[0]
_C_W6 = 1408       # [128, 1]
_CB = 1409

_CACHE = {}


def _bf16(a):
    """fp32 array -> ml_dtypes.bfloat16 (round to nearest even)."""
    import ml_dtypes
    return np.asarray(a, dtype=np.float32).astype(ml_dtypes.bfloat16)


def _build(b1, b6):
    nc = bacc.Bacc("TRN2", target_bir_lowering=False, debug=False,
                   num_devices=N_CORES)

    feat_in = nc.dram_tensor("featsrc", [16, T * BL], BF16,
                             kind="ExternalInput").ap()
    enc_in = nc.dram_tensor("encpack", [F, ENC_TOT * BL], BF16,
                            kind="ExternalInput").ap()
    wb_in = nc.dram_tensor("wpackb", [128, _CB], BF16,
                           kind="ExternalInput").ap()
    y_out = nc.dram_tensor("yout", [1, T * BL], F32,
                           kind="ExternalOutput").ap()

    AF = mybir.ActivationFunctionType
    OP = mybir.AluOpType
    b6f = float(b6[0])

    with tile.TileContext(nc) as tc:
        with (
            tc.tile_pool(name="const", bufs=1) as cp,
            tc.tile_pool(name="us_p", bufs=4) as us_p,
            tc.tile_pool(name="mg_p", bufs=4) as mg_p,
            tc.tile_pool(name="h_p", bufs=2) as h_p,
            tc.tile_pool(name="pd_p", bufs=4, space="PSUM") as pd_p,
            tc.tile_pool(name="po_p", bufs=2, space="PSUM") as po_p,
            tc.tile_pool(name="pin_p", bufs=1, space="PSUM") as pin_p,
            tc.tile_pool(name="ph_p", bufs=1, space="PSUM") as ph_p,
        ):
            featT = cp.tile([16, T * BL], BF16, tag="featT")
            encsb = cp.tile([F, ENC_TOT * BL], BF16, tag="encsb")
            wb = cp.tile([128, _CB], BF16, tag="wb")
            bias = cp.tile([64, 1], F32, tag="bias")
            INS = [cp.tile([F, T * BL], BF16, tag=f"in{i}",
                           name=f"in{i}") for i in range(6)]
            sk = cp.tile([F, 6 * BL], BF16, tag="sk")
            yout_sb = cp.tile([1, T * BL], F32, tag="yout_sb")

            nc.vector.memset(bias[:], 0.0)
            if float(np.abs(b1).max()) != 0.0:
                for r0, v in enumerate(np.asarray(b1, np.float32)):
                    nc.vector.memset(bias[r0:r0 + 1, 0:1], float(v))

            nc.sync.dma_start(featT[:], feat_in[:])
            nc.sync.dma_start(encsb[:], enc_in[:])
            nc.sync.dma_start(wb[:], wb_in[:])

            W2s = wb[0:64, _C_W2:_C_W2 + 128]
            W3s = wb[0:64, _C_W3:_C_W3 + 128]
            M34s = wb[0:64, _C_M34:_C_M34 + 128]
            W4s = wb[0:64, _C_W4:_C_W4 + 128]
            W5s = [wb[0:64, _C_W5 + j * 128:_C_W5 + (j + 1) * 128]
                   for j in range(6)]
            W1s = wb[0:16, _C_W1:_C_W1 + 64]
            As = wb[:, _C_A:_C_A + 64]
            W6s = wb[:, _C_W6:_C_W6 + 1]
            b1s = bias[:, 0:1]

            def blk(t):
                return slice(t * BL, (t + 1) * BL)

            def state_ap(i, t):
                d = DILATIONS[i]
                if t < d:
                    return encsb[:, blk(ENC_OFF[i] + t)]
                return INS[i + 1][:, blk(t - d)]

            # HAM warm-up: dense back-to-back matmuls flip the PE clock
            # gate to 8/8 (2.4 GHz); overlaps the input DMAs.
            wu = pd_p.tile([128, BL], F32, tag="pd", name="warmup")
            for w in range(20):
                nc.tensor.matmul(wu[:], wb[:, 0:128], wb[:, 0:BL],
                                 start=(w == 0), stop=(w == 19))

            pin = pin_p.tile([64, BL], F32, tag="pin", name="pin0")
            nc.tensor.matmul(pin[:], W1s, featT[:, blk(0)],
                             start=True, stop=True)

            pds = {}
            for k in range(4):
                pds[k] = pd_p.tile([128, BL], F32, tag="pd",
                                   name=f"pd0_{k}")
                nc.tensor.matmul(pds[k][:], W2s, state_ap(k, 0),
                                 start=True, stop=False)

            for t in range(T):
                # -- step head: IN0 = tanh(pin).  The W2 births for this
                # step were issued at the previous tail (after A@h) so
                # they execute DURING this tanh and the W3 below runs
                # back-to-back behind them.
                nc.scalar.activation(INS[0][:, blk(t)], pin[:],
                                     AF.Tanh, bias=b1s)
                IN0 = INS[0][:, blk(t)]
                nc.tensor.matmul(pds[0][:], W3s, IN0,
                                 start=False, stop=True)
                nc.tensor.matmul(pds[1][:], W3s, IN0,
                                 start=False, stop=False)

                ph = ph_p.tile([HID, BL], F32, tag="ph", name=f"ph{t}")
                pos = [None] * 6

                def blk_s(j):
                    return slice(j * BL, (j + 1) * BL)

                def relu_skip(j):
                    nc.scalar.activation(sk[:, blk_s(j)], pos[j][0:64, :],
                                         AF.Relu)

                for i in range(6):
                    # deferred relu (j = i-2) first: fills the ACT gap
                    # while pd_i's chain matmul lands.
                    if i >= 2:
                        relu_skip(i - 2)
                    us = us_p.tile([128, 2 * BL], BF16, tag="us")
                    nc.scalar.activation(us[:, 0:BL], pds[i][:], AF.Tanh)
                    uf = us[0:64, 0:BL]
                    # ready fillers -- execute DURING the tanh.
                    if 1 <= i < 5:
                        nc.tensor.matmul(pds[i + 1][:], W3s,
                                         INS[i - 1][:, blk(t)],
                                         start=False, stop=False)
                    if i == 0:
                        pds[4] = pd_p.tile([128, BL], F32, tag="pd",
                                           name=f"pd{t}_4")
                        nc.tensor.matmul(pds[4][:], W2s, state_ap(4, t),
                                         start=True, stop=False)
                    if i == 1:
                        pds[5] = pd_p.tile([128, BL], F32, tag="pd",
                                           name=f"pd{t}_5")
                        nc.tensor.matmul(pds[5][:], W2s, state_ap(5, t),
                                         start=True, stop=False)
                    if i >= 2:
                        j = i - 2
                        nc.tensor.matmul(ph[:], W5s[j], sk[:, blk_s(j)],
                                         start=(j == 0), stop=False)
                    # u_f-gated fillers (mg2 = u_f*u_g + u_f) -- start at
                    # tanh-data, cover the DVE window so the prod halves
                    # run back-to-back.
                    if i < 5:
                        nc.tensor.matmul(pds[i + 1][:], M34s, uf,
                                         start=False, stop=False)
                    po = po_p.tile([128, BL], F32, tag="po",
                                   name=f"po{t}_{i}")
                    nc.tensor.matmul(po[:], W4s, uf, start=True,
                                     stop=False)
                    pos[i] = po
                    # DVE: copy u_g beside u_f (two-tensor DVE ops need
                    # equal base partitions), then the product.
                    nc.vector.tensor_copy(us[0:64, BL:2 * BL],
                                          us[64:128, 0:BL])
                    mg = mg_p.tile([64, BL], BF16, tag="mg")
                    nc.vector.tensor_mul(mg[:], us[0:64, BL:2 * BL], uf)
                    # prod-gated chain group.
                    if i < 5:
                        nc.tensor.matmul(pds[i + 1][:], M34s, mg[:],
                                         start=False, stop=True)
                    nc.tensor.matmul(po[:], W4s, mg[:], start=False,
                                     stop=True)
                    # off-chain telescope pair AFTER the chain group: its
                    # ready half bypass-fills the last pre-chain gap.
                    if i + 2 <= 5:
                        nc.tensor.matmul(pds[i + 2][:], M34s, uf,
                                         start=False, stop=False)
                        nc.tensor.matmul(pds[i + 2][:], M34s, mg[:],
                                         start=False, stop=False)
                    # deferred residual add (j = i-1) off the chain.
                    if i >= 1:
                        j = i - 1
                        nc.vector.tensor_add(INS[j + 1][:, blk(t)],
                                             pos[j][64:128, :],
                                             INS[j][:, blk(t)])

                # -- tail
                relu_skip(4)
                nc.vector.tensor_scalar_max(sk[:, blk_s(5)],
                                            pos[5][0:64, :], 0.0)
                nc.tensor.matmul(ph[:], W5s[4], sk[:, blk_s(4)],
                                 start=False, stop=False)
                nc.tensor.matmul(ph[:], W5s[5], sk[:, blk_s(5)],
                                 start=False, stop=True)
                h = h_p.tile([HID, BL], BF16, tag="h")
                nc.vector.tensor_scalar_max(h[:], ph[:], 0.0)
                if t + 1 < T:
                    pin = pin_p.tile([64, BL], F32, tag="pin",
                                     name=f"pin{t + 1}")
                    nc.tensor.matmul(pin[:], W1s, featT[:, blk(t + 1)],
                                     start=True, stop=False)
                    nc.tensor.matmul(pin[:], As, h[:],
                                     start=False, stop=True)
                py = po_p.tile([1, BL], F32, tag="po", name=f"py{t}")
                nc.tensor.matmul(py[:], W6s, h[:], start=True, stop=True)
                nc.vector.tensor_scalar_add(yout_sb[0:1, blk(t)], py[:],
                                            b6f)
                if t + 1 < T:
                    pds = {}
                    for k in range(4):
                        pds[k] = pd_p.tile([128, BL], F32, tag="pd",
                                           name=f"pd{t + 1}_{k}")
                        nc.tensor.matmul(pds[k][:], W2s,
                                         state_ap(k, t + 1),
                                         start=True, stop=False)

            nc.sync.dma_start(y_out[:], yout_sb[:])

    nc.compile()
    return nc


def _pack_inputs(decoder_features, decoder_init_input, encoder_states,
                 W1, W2, W3, W4, W5, W6):
    """Host-side shard + transpose + bf16-cast.  Returns per-core in_maps."""
    wbp = np.zeros((128, _CB), np.float32)
    wbp[0:64, _C_W2:_C_W2 + 64] = W2[:, 0:64]
    wbp[0:64, _C_W2 + 64:_C_W2 + 128] = 0.5 * W2[:, 64:128]
    wbp[0:64, _C_W3:_C_W3 + 64] = W3[:, 0:64]
    wbp[0:64, _C_W3 + 64:_C_W3 + 128] = 0.5 * W3[:, 64:128]
    M34 = 0.5 * (W4[:, 64:128] @ W3)                 # [64, 128]
    wbp[0:64, _C_M34:_C_M34 + 64] = M34[:, 0:64]
    wbp[0:64, _C_M34 + 64:_C_M34 + 128] = 0.5 * M34[:, 64:128]
    wbp[0:64, _C_W4:_C_W4 + 128] = 0.5 * W4
    for j in range(6):
        wbp[0:64, _C_W5 + j * 128:_C_W5 + (j + 1) * 128] = \
            W5[j * 64:(j + 1) * 64, :]
    wbp[0:16, _C_W1:_C_W1 + 64] = W1
    wbp[:, _C_A:_C_A + 64] = W6 @ W1[0:1, :]         # [128, 64]
    wbp[:, _C_W6:_C_W6 + 1] = W6
    wb_bits = _bf16(wbp)

    in_maps = []
    for c in range(N_CORES):
        s = slice(c * BL, (c + 1) * BL)
        # featT [16, T*BL]: row 0 blk 0 = init, zero elsewhere;
        # rows 1:16 = features^T.
        ft = np.zeros((16, T, BL), np.float32)
        ft[0, 0, :] = decoder_init_input[s, 0]
        ft[1:16] = decoder_features[s].transpose(2, 1, 0)
        # encpack [64, ENC_TOT*BL]
        ep = np.zeros((F, ENC_TOT, BL), np.float32)
        for i, d in enumerate(DILATIONS):
            n = ENC_N[i]
            ep[:, ENC_OFF[i]:ENC_OFF[i] + n, :] = \
                encoder_states[i, s, L - d:L - d + n, :].transpose(2, 1, 0)
        in_maps.append({
            "featsrc": _bf16(ft.reshape(16, T * BL)),
            "encpack": _bf16(ep.reshape(F, ENC_TOT * BL)),
            "wpackb": wb_bits,
        })
    return in_maps


def kernel(**inputs):
    decoder_features = np.asarray(inputs["decoder_features"], np.float32)
    decoder_init_input = np.asarray(inputs["decoder_init_input"], np.float32)
    encoder_states = np.asarray(inputs["encoder_states"], np.float32)
    W1 = np.asarray(inputs["W1"], np.float32)
    b1 = np.asarray(inputs["b1"], np.float32)
    W2 = np.asarray(inputs["W2"], np.float32)
    b2 = np.asarray(inputs["b2"], np.float32)
    W3 = np.asarray(inputs["W3"], np.float32)
    W4 = np.asarray(inputs["W4"], np.float32)
    b4 = np.asarray(inputs["b4"], np.float32)
    W5 = np.asarray(inputs["W5"], np.float32)
    b5 = np.asarray(inputs["b5"], np.float32)
    W6 = np.asarray(inputs["W6"], np.float32)
    b6 = np.asarray(inputs["b6"], np.float32)
    assert float(np.abs(b2).max()) == 0.0, "kernel assumes b2 == 0"
    assert float(np.abs(b4).max()) == 0.0, \
        "kernel's telescoped dilated accumulation assumes b4 == 0"
    assert float(np.abs(b5).max()) == 0.0, "kernel assumes b5 == 0"

    key = "nc"
    if key not in _CACHE:
        _CACHE[key] = _build(b1, b6)
    nc = _CACHE[key]

    in_maps = _pack_inputs(decoder_features, decoder_init_input,
                           encoder_states, W1, W2, W3, W4, W5, W6)
    res = run_bass_kernel_spmd(nc, in_maps, list(range(N_CORES)))

    out = np.empty((B, T, 1), np.float32)
    for c in range(N_CORES):
        y = res.results[c]["yout"].reshape(T, BL)
        out[c * BL:(c + 1) * BL, :, 0] = y.T
    return out


# revision 7
# speedup vs baseline: 1.0896x; 1.0054x over previous
"""WaveNet-style decoder (nn_DecoderV2) on 8 TRN2 NeuronCores.

Strategy: pure data parallel over batch (1024 -> 8 x 128). Per core the
recurrence runs with activations stored transposed [feature, batch] so the
batch lives on the free dim and every dense layer is a TensorE matmul with
stationary weights.

Layout (v2 -- f|g on the PARTITION dim):
  - pd [128, BL] PSUM: rows 0:64 = conv filter preactivation, rows 64:128
    = 0.5*gate preactivation, so a SINGLE K=64, M=128 matmul per dense
    (W2/W3/M34 packed as [64, 128] = [f | 0.5*g]) and a SINGLE [128, BL]
    ACT u = tanh(pd) covering both halves (sigmoid(g) = (tanh(g/2)+1)/2).
  - mg = u_f*(u_g+1) = 2*gated via one DVE scalar_tensor_tensor; W4 and
    the M34 shortcut are pre-scaled by 0.5 to compensate.
  - States are read IN PLACE: layer i's state at step t is IN[i+1] block
    (t-d) for t >= d (the residual outputs ARE the appended state) or the
    pre-packed encoder slice for t < d.  No state-copy DMAs at all.
  - Telescoped pd accumulation (depth 3): pd_k = W2@state_k + W3@IN_{m}
    + sum_{j=m}^{k-1} M34@mg_j with m = max(0, k-3), so the chain only
    waits on one M34 matmul per layer; residual adds (DVE) are deferred
    off the critical path and only feed future states / deferred W3.
  - Feedback tail is folded: pin(t+1) = W1[1:]@feats(t+1) + (W6 (x) W1[0])
    @ h(t), a rank-1 composed weight A, skipping y -> featT -> W1 on the
    chain.  featT row 0 is zero for t >= 1, so the same K=16 W1 matmul
    serves t=0 (row 0 = decoder_init) and the static part for t >= 1.
  - Skips: sk [64, 6*BL]; relu on ACT deferred by 2 layers; W5 as six
    K=64 matmuls accumulating in one PSUM bank.

All matmul operands bf16 (PSUM fp32); b2/b4/b5 asserted zero (b1, b6
honored via ACT bias / DVE scalar).
"""

import numpy as np

import concourse.bacc as bacc
import concourse.mybir as mybir
import concourse.tile as tile
from concourse.bass_utils import run_bass_kernel_spmd

F32 = mybir.dt.float32
BF16 = mybir.dt.bfloat16

N_CORES = 8
B = 1024
BL = B // N_CORES          # 128 batch per core
T = 24
F = 64
HID = 128
DILATIONS = (1, 2, 4, 8, 16, 32)
L = 168
ENC_N = [min(d, T) for d in DILATIONS]
ENC_OFF = np.concatenate([[0], np.cumsum(ENC_N)]).astype(int).tolist()
ENC_TOT = int(np.sum(ENC_N))                     # 55

# wpack (bf16) column layout; [f | 0.5g] pairs live on cols, K on rows.
_C_W2 = 0          # [64, 128]
_C_W3 = 128        # [64, 128]
_C_M34 = 256       # [64, 128]  0.5*(W4r@W3) with g-half halved again
_C_W4 = 384        # [64, 128]  0.5*W4
_C_W5 = 512        # 6 x [64, 128] chunks
_C_W1 = 1280       # [16, 64]
_C_A = 1344        # [128, 64]  W6 (x) W1[0]
_C_W6 = 1408       # [128, 1]
_CB = 1409

_CACHE = {}


def _bf16(a):
    """fp32 array -> ml_dtypes.bfloat16 (round to nearest even)."""
    import ml_dtypes
    return np.asarray(a, dtype=np.float32).astype(ml_dtypes.bfloat16)


def _build(b1, b6):
    nc = bacc.Bacc("TRN2", target_bir_lowering=False, debug=False,
                   num_devices=N_CORES)

    feat_in = nc.dram_tensor("featsrc", [16, T * BL], BF16,
                             kind="ExternalInput").ap()
    enc_in = nc.dram_tensor("encpack", [F, ENC_TOT * BL], BF16,
                            kind="ExternalInput").ap()
    wb_in = nc.dram_tensor("wpackb", [128, _CB], BF16,
                           kind="ExternalInput").ap()
    y_out = nc.dram_tensor("yout", [1, T * BL], F32,
                           kind="ExternalOutput").ap()

    AF = mybir.ActivationFunctionType
    OP = mybir.AluOpType
    b6f = float(b6[0])

    with tile.TileContext(nc) as tc:
        with (
            tc.tile_pool(name="const", bufs=1) as cp,
            tc.tile_pool(name="us_p", bufs=4) as us_p,
            tc.tile_pool(name="mg_p", bufs=4) as mg_p,
            tc.tile_pool(name="h_p", bufs=2) as h_p,
            tc.tile_pool(name="pd_p", bufs=4, space="PSUM") as pd_p,
            tc.tile_pool(name="po_p", bufs=2, space="PSUM") as po_p,
            tc.tile_pool(name="pin_p", bufs=1, space="PSUM") as pin_p,
            tc.tile_pool(name="ph_p", bufs=1, space="PSUM") as ph_p,
        ):
            featT = cp.tile([16, T * BL], BF16, tag="featT")
            encsb = cp.tile([F, ENC_TOT * BL], BF16, tag="encsb")
            wb = cp.tile([128, _CB], BF16, tag="wb")
            bias = cp.tile([64, 1], F32, tag="bias")
            INS = [cp.tile([F, T * BL], BF16, tag=f"in{i}",
                           name=f"in{i}") for i in range(6)]
            sk = cp.tile([F, 6 * BL], BF16, tag="sk")
            yout_sb = cp.tile([1, T * BL], F32, tag="yout_sb")

            nc.vector.memset(bias[:], 0.0)
            if float(np.abs(b1).max()) != 0.0:
                for r0, v in enumerate(np.asarray(b1, np.float32)):
                    nc.vector.memset(bias[r0:r0 + 1, 0:1], float(v))

            nc.sync.dma_start(wb[:], wb_in[:])
            nc.sync.dma_start(featT[:], feat_in[:])
            nc.gpsimd.dma_start(encsb[:], enc_in[:])

            W2s = wb[0:64, _C_W2:_C_W2 + 128]
            W3s = wb[0:64, _C_W3:_C_W3 + 128]
            M34s = wb[0:64, _C_M34:_C_M34 + 128]
            W4s = wb[0:64, _C_W4:_C_W4 + 128]
            W5s = [wb[0:64, _C_W5 + j * 128:_C_W5 + (j + 1) * 128]
                   for j in range(6)]
            W1s = wb[0:16, _C_W1:_C_W1 + 64]
            As = wb[:, _C_A:_C_A + 64]
            W6s = wb[:, _C_W6:_C_W6 + 1]
            b1s = bias[:, 0:1]

            def blk(t):
                return slice(t * BL, (t + 1) * BL)

            def state_ap(i, t):
                d = DILATIONS[i]
                if t < d:
                    return encsb[:, blk(ENC_OFF[i] + t)]
                return INS[i + 1][:, blk(t - d)]

            # HAM warm-up: dense back-to-back matmuls flip the PE clock
            # gate to 8/8 (2.4 GHz); reads a memset scratch tile so it
            # runs concurrently with (not after) the input DMAs.
            wsrc = us_p.tile([128, 2 * BL], BF16, tag="us", name="wsrc")
            nc.vector.memset(wsrc[:], 0.25)
            wu = pd_p.tile([128, BL], F32, tag="pd", name="warmup")
            for w in range(20):
                nc.tensor.matmul(wu[:], wsrc[:, 0:128], wsrc[:, 0:BL],
                                 start=(w == 0), stop=(w == 19))

            pin = pin_p.tile([64, BL], F32, tag="pin", name="pin0")
            nc.tensor.matmul(pin[:], W1s, featT[:, blk(0)],
                             start=True, stop=True)

            pds = {}
            for k in range(4):
                pds[k] = pd_p.tile([128, BL], F32, tag="pd",
                                   name=f"pd0_{k}")
                nc.tensor.matmul(pds[k][:], W2s, state_ap(k, 0),
                                 start=True, stop=False)

            for t in range(T):
                # -- step head: IN0 = tanh(pin).  The W2 births for this
                # step were issued at the previous tail (after A@h) so
                # they execute DURING this tanh and the W3 below runs
                # back-to-back behind them.
                nc.scalar.activation(INS[0][:, blk(t)], pin[:],
                                     AF.Tanh, bias=b1s)
                IN0 = INS[0][:, blk(t)]
                nc.tensor.matmul(pds[0][:], W3s, IN0,
                                 start=False, stop=True)
                nc.tensor.matmul(pds[1][:], W3s, IN0,
                                 start=False, stop=False)

                ph = ph_p.tile([HID, BL], F32, tag="ph", name=f"ph{t}")
                pos = [None] * 6

                def blk_s(j):
                    return slice(j * BL, (j + 1) * BL)

                def relu_skip(j):
                    nc.scalar.activation(sk[:, blk_s(j)], pos[j][0:64, :],
                                         AF.Relu)

                for i in range(6):
                    # deferred relu (j = i-2) first: fills the ACT gap
                    # while pd_i's chain matmul lands.
                    if i >= 2:
                        relu_skip(i - 2)
                    us = us_p.tile([128, 2 * BL], BF16, tag="us")
                    nc.scalar.activation(us[:, 0:BL], pds[i][:], AF.Tanh)
                    uf = us[0:64, 0:BL]
                    # ready fillers -- execute DURING the tanh.
                    if 1 <= i < 5:
                        nc.tensor.matmul(pds[i + 1][:], W3s,
                                         INS[i - 1][:, blk(t)],
                                         start=False, stop=False)
                    if i == 0:
                        pds[4] = pd_p.tile([128, BL], F32, tag="pd",
                                           name=f"pd{t}_4")
                        nc.tensor.matmul(pds[4][:], W2s, state_ap(4, t),
                                         start=True, stop=False)
                    if i == 1:
                        pds[5] = pd_p.tile([128, BL], F32, tag="pd",
                                           name=f"pd{t}_5")
                        nc.tensor.matmul(pds[5][:], W2s, state_ap(5, t),
                                         start=True, stop=False)
                    if i >= 2:
                        j = i - 2
                        nc.tensor.matmul(ph[:], W5s[j], sk[:, blk_s(j)],
                                         start=(j == 0), stop=False)
                    # u_f-gated fillers (mg2 = u_f*u_g + u_f) -- start at
                    # tanh-data, cover the DVE window so the prod halves
                    # run back-to-back.
                    if i < 5:
                        nc.tensor.matmul(pds[i + 1][:], M34s, uf,
                                         start=False, stop=False)
                    po = po_p.tile([128, BL], F32, tag="po",
                                   name=f"po{t}_{i}")
                    nc.tensor.matmul(po[:], W4s, uf, start=True,
                                     stop=False)
                    pos[i] = po
                    # DVE: copy u_g beside u_f (two-tensor DVE ops need
                    # equal base partitions), then the product.
                    nc.vector.tensor_copy(us[0:64, BL:2 * BL],
                                          us[64:128, 0:BL])
                    mg = mg_p.tile([64, BL], BF16, tag="mg")
                    nc.vector.tensor_mul(mg[:], us[0:64, BL:2 * BL], uf)
                    # prod-gated chain group.
                    if i < 5:
                        nc.tensor.matmul(pds[i + 1][:], M34s, mg[:],
                                         start=False, stop=True)
                    nc.tensor.matmul(po[:], W4s, mg[:], start=False,
                                     stop=True)
                    # off-chain telescope pair AFTER the chain group: its
                    # ready half bypass-fills the last pre-chain gap.
                    if i + 2 <= 5:
                        nc.tensor.matmul(pds[i + 2][:], M34s, uf,
                                         start=False, stop=False)
                        nc.tensor.matmul(pds[i + 2][:], M34s, mg[:],
                                         start=False, stop=False)
                    # deferred residual add (j = i-1) off the chain.
                    if i >= 1:
                        j = i - 1
                        nc.vector.tensor_add(INS[j + 1][:, blk(t)],
                                             pos[j][64:128, :],
                                             INS[j][:, blk(t)])

                # -- tail
                relu_skip(4)
                nc.vector.tensor_scalar_max(sk[:, blk_s(5)],
                                            pos[5][0:64, :], 0.0)
                nc.tensor.matmul(ph[:], W5s[4], sk[:, blk_s(4)],
                                 start=False, stop=False)
                nc.tensor.matmul(ph[:], W5s[5], sk[:, blk_s(5)],
                                 start=False, stop=True)
                h = h_p.tile([HID, BL], BF16, tag="h")
                nc.vector.tensor_scalar_max(h[:], ph[:], 0.0)
                if t + 1 < T:
                    pin = pin_p.tile([64, BL], F32, tag="pin",
                                     name=f"pin{t + 1}")
                    nc.tensor.matmul(pin[:], W1s, featT[:, blk(t + 1)],
                                     start=True, stop=False)
                    nc.tensor.matmul(pin[:], As, h[:],
                                     start=False, stop=True)
                py = po_p.tile([1, BL], F32, tag="po", name=f"py{t}")
                nc.tensor.matmul(py[:], W6s, h[:], start=True, stop=True)
                nc.vector.tensor_scalar_add(yout_sb[0:1, blk(t)], py[:],
                                            b6f)
                if t % 6 == 5:
                    c0 = (t - 5) * BL
                    c1 = (t + 1) * BL
                    nc.sync.dma_start(y_out[0:1, c0:c1],
                                      yout_sb[0:1, c0:c1])
                if t + 1 < T:
                    pds = {}
                    for k in range(4):
                        pds[k] = pd_p.tile([128, BL], F32, tag="pd",
                                           name=f"pd{t + 1}_{k}")
                        nc.tensor.matmul(pds[k][:], W2s,
                                         state_ap(k, t + 1),
                                         start=True, stop=False)


    nc.compile()
    return nc


def _pack_inputs(decoder_features, decoder_init_input, encoder_states,
                 W1, W2, W3, W4, W5, W6):
    """Host-side shard + transpose + bf16-cast.  Returns per-core in_maps."""
    wbp = np.zeros((128, _CB), np.float32)
    wbp[0:64, _C_W2:_C_W2 + 64] = W2[:, 0:64]
    wbp[0:64, _C_W2 + 64:_C_W2 + 128] = 0.5 * W2[:, 64:128]
    wbp[0:64, _C_W3:_C_W3 + 64] = W3[:, 0:64]
    wbp[0:64, _C_W3 + 64:_C_W3 + 128] = 0.5 * W3[:, 64:128]
    M34 = 0.5 * (W4[:, 64:128] @ W3)                 # [64, 128]
    wbp[0:64, _C_M34:_C_M34 + 64] = M34[:, 0:64]
    wbp[0:64, _C_M34 + 64:_C_M34 + 128] = 0.5 * M34[:, 64:128]
    wbp[0:64, _C_W4:_C_W4 + 128] = 0.5 * W4
    for j in range(6):
        wbp[0:64, _C_W5 + j * 128:_C_W5 + (j + 1) * 128] = \
            W5[j * 64:(j + 1) * 64, :]
    wbp[0:16, _C_W1:_C_W1 + 64] = W1
    wbp[:, _C_A:_C_A + 64] = W6 @ W1[0:1, :]         # [128, 64]
    wbp[:, _C_W6:_C_W6 + 1] = W6
    wb_bits = _bf16(wbp)

    in_maps = []
    for c in range(N_CORES):
        s = slice(c * BL, (c + 1) * BL)
        # featT [16, T*BL]: row 0 blk 0 = init, zero elsewhere;
        # rows 1:16 = features^T.
        ft = np.zeros((16, T, BL), np.float32)
        ft[0, 0, :] = decoder_init_input[s, 0]
        ft[1:16] = decoder_features[s].transpose(2, 1, 0)
        # encpack [64, ENC_TOT*BL]
        ep = np.zeros((F, ENC_TOT, BL), np.float32)
        for i, d in enumerate(DILATIONS):
            n = ENC_N[i]
            ep[:, ENC_OFF[i]:ENC_OFF[i] + n, :] = \
                encoder_states[i, s, L - d:L - d + n, :].transpose(2, 1, 0)
        in_maps.append({
            "featsrc": _bf16(ft.reshape(16, T * BL)),
            "encpack": _bf16(ep.reshape(F, ENC_TOT * BL)),
            "wpackb": wb_bits,
        })
    return in_maps


def kernel(**inputs):
    decoder_features = np.asarray(inputs["decoder_features"], np.float32)
    decoder_init_input = np.asarray(inputs["decoder_init_input"], np.float32)
    encoder_states = np.asarray(inputs["encoder_states"], np.float32)
    W1 = np.asarray(inputs["W1"], np.float32)
    b1 = np.asarray(inputs["b1"], np.float32)
    W2 = np.asarray(inputs["W2"], np.float32)
    b2 = np.asarray(inputs["b2"], np.float32)
    W3 = np.asarray(inputs["W3"], np.float32)
    W4 = np.asarray(inputs["W4"], np.float32)
    b4 = np.asarray(inputs["b4"], np.float32)
    W5 = np.asarray(inputs["W5"], np.float32)
    b5 = np.asarray(inputs["b5"], np.float32)
    W6 = np.asarray(inputs["W6"], np.float32)
    b6 = np.asarray(inputs["b6"], np.float32)
    assert float(np.abs(b2).max()) == 0.0, "kernel assumes b2 == 0"
    assert float(np.abs(b4).max()) == 0.0, \
        "kernel's telescoped dilated accumulation assumes b4 == 0"
    assert float(np.abs(b5).max()) == 0.0, "kernel assumes b5 == 0"

    key = "nc"
    if key not in _CACHE:
        _CACHE[key] = _build(b1, b6)
    nc = _CACHE[key]

    in_maps = _pack_inputs(decoder_features, decoder_init_input,
                           encoder_states, W1, W2, W3, W4, W5, W6)
    res = run_bass_kernel_spmd(nc, in_maps, list(range(N_CORES)))

    out = np.empty((B, T, 1), np.float32)
    for c in range(N_CORES):
        y = res.results[c]["yout"].reshape(T, BL)
        out[c * BL:(c + 1) * BL, :, 0] = y.T
    return out


# revision 8
# speedup vs baseline: 1.0897x; 1.0001x over previous
"""WaveNet-style decoder (nn_DecoderV2) on 8 TRN2 NeuronCores.

Strategy: pure data parallel over batch (1024 -> 8 x 128). Per core the
recurrence runs with activations stored transposed [feature, batch] so the
batch lives on the free dim and every dense layer is a TensorE matmul with
stationary weights.

Layout (v2 -- f|g on the PARTITION dim):
  - pd [128, BL] PSUM: rows 0:64 = conv filter preactivation, rows 64:128
    = 0.5*gate preactivation, so a SINGLE K=64, M=128 matmul per dense
    (W2/W3/M34 packed as [64, 128] = [f | 0.5*g]) and a SINGLE [128, BL]
    ACT u = tanh(pd) covering both halves (sigmoid(g) = (tanh(g/2)+1)/2).
  - mg = u_f*(u_g+1) = 2*gated via one DVE scalar_tensor_tensor; W4 and
    the M34 shortcut are pre-scaled by 0.5 to compensate.
  - States are read IN PLACE: layer i's state at step t is IN[i+1] block
    (t-d) for t >= d (the residual outputs ARE the appended state) or the
    pre-packed encoder slice for t < d.  No state-copy DMAs at all.
  - Telescoped pd accumulation (depth 3): pd_k = W2@state_k + W3@IN_{m}
    + sum_{j=m}^{k-1} M34@mg_j with m = max(0, k-3), so the chain only
    waits on one M34 matmul per layer; residual adds (DVE) are deferred
    off the critical path and only feed future states / deferred W3.
  - Feedback tail is folded: pin(t+1) = W1[1:]@feats(t+1) + (W6 (x) W1[0])
    @ h(t), a rank-1 composed weight A, skipping y -> featT -> W1 on the
    chain.  featT row 0 is zero for t >= 1, so the same K=16 W1 matmul
    serves t=0 (row 0 = decoder_init) and the static part for t >= 1.
  - Skips: sk [64, 6*BL]; relu on ACT deferred by 2 layers; W5 as six
    K=64 matmuls accumulating in one PSUM bank.

All matmul operands bf16 (PSUM fp32); b2/b4/b5 asserted zero (b1, b6
honored via ACT bias / DVE scalar).
"""

import numpy as np

import concourse.bacc as bacc
import concourse.mybir as mybir
import concourse.tile as tile
from concourse.bass_utils import run_bass_kernel_spmd

F32 = mybir.dt.float32
BF16 = mybir.dt.bfloat16

N_CORES = 8
B = 1024
BL = B // N_CORES          # 128 batch per core
T = 24
F = 64
HID = 128
DILATIONS = (1, 2, 4, 8, 16, 32)
L = 168
ENC_N = [min(d, T) for d in DILATIONS]
ENC_OFF = np.concatenate([[0], np.cumsum(ENC_N)]).astype(int).tolist()
ENC_TOT = int(np.sum(ENC_N))                     # 55

# wpack (bf16) column layout; [f | 0.5g] pairs live on cols, K on rows.
_C_W2 = 0          # [64, 128]
_C_W3 = 128        # [64, 128]
_C_M34 = 256       # [64, 128]  0.5*(W4r@W3) with g-half halved again
_C_W4 = 384        # [64, 128]  0.5*W4
_C_W5 = 512        # 6 x [64, 128] chunks
_C_W1 = 1280       # [16, 64]
_C_A = 1344        # [128, 64]  W6 (x) W1[0]
_C_W6 = 1408       # [128, 1]
_CB = 1409

_CACHE = {}


def _bf16(a):
    """fp32 array -> ml_dtypes.bfloat16 (round to nearest even)."""
    import ml_dtypes
    return np.asarray(a, dtype=np.float32).astype(ml_dtypes.bfloat16)


def _build(b1, b6):
    nc = bacc.Bacc("TRN2", target_bir_lowering=False, debug=False,
                   num_devices=N_CORES)

    feat_in = nc.dram_tensor("featsrc", [16, T * BL], BF16,
                             kind="ExternalInput").ap()
    enc_in = nc.dram_tensor("encpack", [F, ENC_TOT * BL], BF16,
                            kind="ExternalInput").ap()
    wb_in = nc.dram_tensor("wpackb", [128, _CB], BF16,
                           kind="ExternalInput").ap()
    y_out = nc.dram_tensor("yout", [1, T * BL], F32,
                           kind="ExternalOutput").ap()

    AF = mybir.ActivationFunctionType
    OP = mybir.AluOpType
    b6f = float(b6[0])

    with tile.TileContext(nc) as tc:
        with (
            tc.tile_pool(name="const", bufs=1) as cp,
            tc.tile_pool(name="us_p", bufs=4) as us_p,
            tc.tile_pool(name="mg_p", bufs=4) as mg_p,
            tc.tile_pool(name="h_p", bufs=2) as h_p,
            tc.tile_pool(name="pd_p", bufs=4, space="PSUM") as pd_p,
            tc.tile_pool(name="po_p", bufs=2, space="PSUM") as po_p,
            tc.tile_pool(name="pin_p", bufs=1, space="PSUM") as pin_p,
            tc.tile_pool(name="ph_p", bufs=1, space="PSUM") as ph_p,
        ):
            featT = cp.tile([16, T * BL], BF16, tag="featT")
            encsb = cp.tile([F, ENC_TOT * BL], BF16, tag="encsb")
            wb = cp.tile([128, _CB], BF16, tag="wb")
            bias = cp.tile([64, 1], F32, tag="bias")
            INS = [cp.tile([F, T * BL], BF16, tag=f"in{i}",
                           name=f"in{i}") for i in range(6)]
            sk = cp.tile([F, 6 * BL], BF16, tag="sk")
            yout_sb = cp.tile([1, T * BL], F32, tag="yout_sb")

            nc.vector.memset(bias[:], 0.0)
            if float(np.abs(b1).max()) != 0.0:
                for r0, v in enumerate(np.asarray(b1, np.float32)):
                    nc.vector.memset(bias[r0:r0 + 1, 0:1], float(v))

            nc.sync.dma_start(wb[:], wb_in[:])
            nc.sync.dma_start(featT[:], feat_in[:])
            nc.gpsimd.dma_start(encsb[:], enc_in[:])

            W2s = wb[0:64, _C_W2:_C_W2 + 128]
            W3s = wb[0:64, _C_W3:_C_W3 + 128]
            M34s = wb[0:64, _C_M34:_C_M34 + 128]
            W4s = wb[0:64, _C_W4:_C_W4 + 128]
            W5s = [wb[0:64, _C_W5 + j * 128:_C_W5 + (j + 1) * 128]
                   for j in range(6)]
            W1s = wb[0:16, _C_W1:_C_W1 + 64]
            As = wb[:, _C_A:_C_A + 64]
            W6s = wb[:, _C_W6:_C_W6 + 1]
            b1s = bias[:, 0:1]

            def blk(t):
                return slice(t * BL, (t + 1) * BL)

            def state_ap(i, t):
                d = DILATIONS[i]
                if t < d:
                    return encsb[:, blk(ENC_OFF[i] + t)]
                return INS[i + 1][:, blk(t - d)]

            # HAM warm-up: dense back-to-back matmuls flip the PE clock
            # gate to 8/8 (2.4 GHz); reads a memset scratch tile so it
            # runs concurrently with (not after) the input DMAs.
            wsrc = us_p.tile([128, 2 * BL], BF16, tag="us", name="wsrc")
            nc.vector.memset(wsrc[:], 0.25)
            wu = pd_p.tile([128, BL], F32, tag="pd", name="warmup")
            for w in range(20):
                nc.tensor.matmul(wu[:], wsrc[:, 0:128], wsrc[:, 0:BL],
                                 start=(w == 0), stop=(w == 19))

            pin = pin_p.tile([64, BL], F32, tag="pin", name="pin0")
            nc.tensor.matmul(pin[:], W1s, featT[:, blk(0)],
                             start=True, stop=True)

            pds = {}
            for k in range(4):
                pds[k] = pd_p.tile([128, BL], F32, tag="pd",
                                   name=f"pd0_{k}")
                nc.tensor.matmul(pds[k][:], W2s, state_ap(k, 0),
                                 start=True, stop=False)

            for t in range(T):
                # -- step head: IN0 = tanh(pin).  The W2 births for this
                # step were issued at the previous tail (after A@h) so
                # they execute DURING this tanh and the W3 below runs
                # back-to-back behind them.
                nc.scalar.activation(INS[0][:, blk(t)], pin[:],
                                     AF.Tanh, bias=b1s)
                IN0 = INS[0][:, blk(t)]
                nc.tensor.matmul(pds[0][:], W3s, IN0,
                                 start=False, stop=True)
                nc.tensor.matmul(pds[1][:], W3s, IN0,
                                 start=False, stop=False)

                ph = ph_p.tile([HID, BL], F32, tag="ph", name=f"ph{t}")
                pos = [None] * 6
                pds_n = {}

                def blk_s(j):
                    return slice(j * BL, (j + 1) * BL)

                def relu_skip(j):
                    nc.scalar.activation(sk[:, blk_s(j)], pos[j][0:64, :],
                                         AF.Relu)

                for i in range(6):
                    # deferred relu (j = i-2) first: fills the ACT gap
                    # while pd_i's chain matmul lands.
                    if i >= 2:
                        relu_skip(i - 2)
                    us = us_p.tile([128, 2 * BL], BF16, tag="us")
                    nc.scalar.activation(us[:, 0:BL], pds[i][:], AF.Tanh)
                    uf = us[0:64, 0:BL]
                    # ready fillers -- execute DURING the tanh.
                    if 1 <= i < 5:
                        nc.tensor.matmul(pds[i + 1][:], W3s,
                                         INS[i - 1][:, blk(t)],
                                         start=False, stop=False)
                    if i == 0:
                        pds[4] = pd_p.tile([128, BL], F32, tag="pd",
                                           name=f"pd{t}_4")
                        nc.tensor.matmul(pds[4][:], W2s, state_ap(4, t),
                                         start=True, stop=False)
                    if i == 1:
                        pds[5] = pd_p.tile([128, BL], F32, tag="pd",
                                           name=f"pd{t}_5")
                        nc.tensor.matmul(pds[5][:], W2s, state_ap(5, t),
                                         start=True, stop=False)
                    if i >= 2:
                        j = i - 2
                        nc.tensor.matmul(ph[:], W5s[j], sk[:, blk_s(j)],
                                         start=(j == 0), stop=False)
                    # u_f-gated fillers (mg2 = u_f*u_g + u_f) -- start at
                    # tanh-data, cover the DVE window so the prod halves
                    # run back-to-back.
                    if i < 5:
                        nc.tensor.matmul(pds[i + 1][:], M34s, uf,
                                         start=False, stop=False)
                    po = po_p.tile([128, BL], F32, tag="po",
                                   name=f"po{t}_{i}")
                    nc.tensor.matmul(po[:], W4s, uf, start=True,
                                     stop=False)
                    pos[i] = po
                    if i == 5 and t + 1 < T:
                        for k in (0, 1):
                            pds_n[k] = pd_p.tile(
                                [128, BL], F32, tag="pd",
                                name=f"pd{t + 1}_{k}")
                            nc.tensor.matmul(pds_n[k][:], W2s,
                                             state_ap(k, t + 1),
                                             start=True, stop=False)
                    # DVE: copy u_g beside u_f (two-tensor DVE ops need
                    # equal base partitions), then the product.
                    nc.vector.tensor_copy(us[0:64, BL:2 * BL],
                                          us[64:128, 0:BL])
                    mg = mg_p.tile([64, BL], BF16, tag="mg")
                    nc.vector.tensor_mul(mg[:], us[0:64, BL:2 * BL], uf)
                    # prod-gated chain group.
                    if i < 5:
                        nc.tensor.matmul(pds[i + 1][:], M34s, mg[:],
                                         start=False, stop=True)
                    nc.tensor.matmul(po[:], W4s, mg[:], start=False,
                                     stop=True)
                    # off-chain telescope pair AFTER the chain group: its
                    # ready half bypass-fills the last pre-chain gap.
                    if i + 2 <= 5:
                        nc.tensor.matmul(pds[i + 2][:], M34s, uf,
                                         start=False, stop=False)
                        nc.tensor.matmul(pds[i + 2][:], M34s, mg[:],
                                         start=False, stop=False)
                    # deferred residual add (j = i-1) off the chain.
                    if i >= 1:
                        j = i - 1
                        nc.vector.tensor_add(INS[j + 1][:, blk(t)],
                                             pos[j][64:128, :],
                                             INS[j][:, blk(t)])

                # -- tail
                relu_skip(4)
                nc.vector.tensor_scalar_max(sk[:, blk_s(5)],
                                            pos[5][0:64, :], 0.0)
                nc.tensor.matmul(ph[:], W5s[4], sk[:, blk_s(4)],
                                 start=False, stop=False)
                nc.tensor.matmul(ph[:], W5s[5], sk[:, blk_s(5)],
                                 start=False, stop=True)
                h = h_p.tile([HID, BL], BF16, tag="h")
                nc.vector.tensor_scalar_max(h[:], ph[:], 0.0)
                if t + 1 < T:
                    pin = pin_p.tile([64, BL], F32, tag="pin",
                                     name=f"pin{t + 1}")
                    nc.tensor.matmul(pin[:], W1s, featT[:, blk(t + 1)],
                                     start=True, stop=False)
                    nc.tensor.matmul(pin[:], As, h[:],
                                     start=False, stop=True)
                py = po_p.tile([1, BL], F32, tag="po", name=f"py{t}")
                nc.tensor.matmul(py[:], W6s, h[:], start=True, stop=True)
                nc.vector.tensor_scalar_add(yout_sb[0:1, blk(t)], py[:],
                                            b6f)
                if t % 6 == 5:
                    c0 = (t - 5) * BL
                    c1 = (t + 1) * BL
                    nc.sync.dma_start(y_out[0:1, c0:c1],
                                      yout_sb[0:1, c0:c1])
                if t + 1 < T:
                    for k in (2, 3):
                        pds_n[k] = pd_p.tile([128, BL], F32, tag="pd",
                                             name=f"pd{t + 1}_{k}")
                        nc.tensor.matmul(pds_n[k][:], W2s,
                                         state_ap(k, t + 1),
                                         start=True, stop=False)
                    pds = pds_n


    nc.compile()
    return nc


def _pack_inputs(decoder_features, decoder_init_input, encoder_states,
                 W1, W2, W3, W4, W5, W6):
    """Host-side shard + transpose + bf16-cast.  Returns per-core in_maps."""
    wbp = np.zeros((128, _CB), np.float32)
    wbp[0:64, _C_W2:_C_W2 + 64] = W2[:, 0:64]
    wbp[0:64, _C_W2 + 64:_C_W2 + 128] = 0.5 * W2[:, 64:128]
    wbp[0:64, _C_W3:_C_W3 + 64] = W3[:, 0:64]
    wbp[0:64, _C_W3 + 64:_C_W3 + 128] = 0.5 * W3[:, 64:128]
    M34 = 0.5 * (W4[:, 64:128] @ W3)                 # [64, 128]
    wbp[0:64, _C_M34:_C_M34 + 64] = M34[:, 0:64]
    wbp[0:64, _C_M34 + 64:_C_M34 + 128] = 0.5 * M34[:, 64:128]
    wbp[0:64, _C_W4:_C_W4 + 128] = 0.5 * W4
    for j in range(6):
        wbp[0:64, _C_W5 + j * 128:_C_W5 + (j + 1) * 128] = \
            W5[j * 64:(j + 1) * 64, :]
    wbp[0:16, _C_W1:_C_W1 + 64] = W1
    wbp[:, _C_A:_C_A + 64] = W6 @ W1[0:1, :]         # [128, 64]
    wbp[:, _C_W6:_C_W6 + 1] = W6
    wb_bits = _bf16(wbp)

    in_maps = []
    for c in range(N_CORES):
        s = slice(c * BL, (c + 1) * BL)
        # featT [16, T*BL]: row 0 blk 0 = init, zero elsewhere;
        # rows 1:16 = features^T.
        ft = np.zeros((16, T, BL), np.float32)
        ft[0, 0, :] = decoder_init_input[s, 0]
        ft[1:16] = decoder_features[s].transpose(2, 1, 0)
        # encpack [64, ENC_TOT*BL]
        ep = np.zeros((F, ENC_TOT, BL), np.float32)
        for i, d in enumerate(DILATIONS):
            n = ENC_N[i]
            ep[:, ENC_OFF[i]:ENC_OFF[i] + n, :] = \
                encoder_states[i, s, L - d:L - d + n, :].transpose(2, 1, 0)
        in_maps.append({
            "featsrc": _bf16(ft.reshape(16, T * BL)),
            "encpack": _bf16(ep.reshape(F, ENC_TOT * BL)),
            "wpackb": wb_bits,
        })
    return in_maps


def kernel(**inputs):
    decoder_features = np.asarray(inputs["decoder_features"], np.float32)
    decoder_init_input = np.asarray(inputs["decoder_init_input"], np.float32)
    encoder_states = np.asarray(inputs["encoder_states"], np.float32)
    W1 = np.asarray(inputs["W1"], np.float32)
    b1 = np.asarray(inputs["b1"], np.float32)
    W2 = np.asarray(inputs["W2"], np.float32)
    b2 = np.asarray(inputs["b2"], np.float32)
    W3 = np.asarray(inputs["W3"], np.float32)
    W4 = np.asarray(inputs["W4"], np.float32)
    b4 = np.asarray(inputs["b4"], np.float32)
    W5 = np.asarray(inputs["W5"], np.float32)
    b5 = np.asarray(inputs["b5"], np.float32)
    W6 = np.asarray(inputs["W6"], np.float32)
    b6 = np.asarray(inputs["b6"], np.float32)
    assert float(np.abs(b2).max()) == 0.0, "kernel assumes b2 == 0"
    assert float(np.abs(b4).max()) == 0.0, \
        "kernel's telescoped dilated accumulation assumes b4 == 0"
    assert float(np.abs(b5).max()) == 0.0, "kernel assumes b5 == 0"

    key = "nc"
    if key not in _CACHE:
        _CACHE[key] = _build(b1, b6)
    nc = _CACHE[key]

    in_maps = _pack_inputs(decoder_features, decoder_init_input,
                           encoder_states, W1, W2, W3, W4, W5, W6)
    res = run_bass_kernel_spmd(nc, in_maps, list(range(N_CORES)))

    out = np.empty((B, T, 1), np.float32)
    for c in range(N_CORES):
        y = res.results[c]["yout"].reshape(T, BL)
        out[c * BL:(c + 1) * BL, :, 0] = y.T
    return out


# revision 9
# speedup vs baseline: 1.0905x; 1.0007x over previous
"""WaveNet-style decoder (nn_DecoderV2) on 8 TRN2 NeuronCores.

Strategy: pure data parallel over batch (1024 -> 8 x 128). Per core the
recurrence runs with activations stored transposed [feature, batch] so the
batch lives on the free dim and every dense layer is a TensorE matmul with
stationary weights.

Layout (v2 -- f|g on the PARTITION dim):
  - pd [128, BL] PSUM: rows 0:64 = conv filter preactivation, rows 64:128
    = 0.5*gate preactivation, so a SINGLE K=64, M=128 matmul per dense
    (W2/W3/M34 packed as [64, 128] = [f | 0.5*g]) and a SINGLE [128, BL]
    ACT u = tanh(pd) covering both halves (sigmoid(g) = (tanh(g/2)+1)/2).
  - mg = u_f*(u_g+1) = 2*gated via one DVE scalar_tensor_tensor; W4 and
    the M34 shortcut are pre-scaled by 0.5 to compensate.
  - States are read IN PLACE: layer i's state at step t is IN[i+1] block
    (t-d) for t >= d (the residual outputs ARE the appended state) or the
    pre-packed encoder slice for t < d.  No state-copy DMAs at all.
  - Telescoped pd accumulation (depth 3): pd_k = W2@state_k + W3@IN_{m}
    + sum_{j=m}^{k-1} M34@mg_j with m = max(0, k-3), so the chain only
    waits on one M34 matmul per layer; residual adds (DVE) are deferred
    off the critical path and only feed future states / deferred W3.
  - Feedback tail is folded: pin(t+1) = W1[1:]@feats(t+1) + (W6 (x) W1[0])
    @ h(t), a rank-1 composed weight A, skipping y -> featT -> W1 on the
    chain.  featT row 0 is zero for t >= 1, so the same K=16 W1 matmul
    serves t=0 (row 0 = decoder_init) and the static part for t >= 1.
  - Skips: sk [64, 6*BL]; relu on ACT deferred by 2 layers; W5 as six
    K=64 matmuls accumulating in one PSUM bank.

All matmul operands bf16 (PSUM fp32); b2/b4/b5 asserted zero (b1, b6
honored via ACT bias / DVE scalar).
"""

import numpy as np

import concourse.bacc as bacc
import concourse.mybir as mybir
import concourse.tile as tile
from concourse.bass_utils import run_bass_kernel_spmd

F32 = mybir.dt.float32
BF16 = mybir.dt.bfloat16

N_CORES = 8
B = 1024
BL = B // N_CORES          # 128 batch per core
T = 24
F = 64
HID = 128
DILATIONS = (1, 2, 4, 8, 16, 32)
L = 168
ENC_N = [min(d, T) for d in DILATIONS]
ENC_OFF = np.concatenate([[0], np.cumsum(ENC_N)]).astype(int).tolist()
ENC_TOT = int(np.sum(ENC_N))                     # 55

# wpack (bf16) column layout; [f | 0.5g] pairs live on cols, K on rows.
_C_W2 = 0          # [64, 128]
_C_W3 = 128        # [64, 128]
_C_M34 = 256       # [64, 128]  0.5*(W4r@W3) with g-half halved again
_C_W4 = 384        # [64, 128]  0.5*W4
_C_W5 = 512        # 6 x [64, 128] chunks
_C_W1 = 1280       # [16, 64]
_C_A = 1344        # [128, 64]  W6 (x) W1[0]
_C_W6 = 1408       # [128, 1]
_CB = 1409

_CACHE = {}


def _bf16(a):
    """fp32 array -> ml_dtypes.bfloat16 (round to nearest even)."""
    import ml_dtypes
    return np.asarray(a, dtype=np.float32).astype(ml_dtypes.bfloat16)


def _build(b1, b6):
    nc = bacc.Bacc("TRN2", target_bir_lowering=False, debug=False,
                   num_devices=N_CORES)

    feat_in = nc.dram_tensor("featsrc", [16, T * BL], BF16,
                             kind="ExternalInput").ap()
    enc_in = nc.dram_tensor("encpack", [F, ENC_TOT * BL], BF16,
                            kind="ExternalInput").ap()
    wb_in = nc.dram_tensor("wpackb", [128, _CB], BF16,
                           kind="ExternalInput").ap()
    y_out = nc.dram_tensor("yout", [1, T * BL], F32,
                           kind="ExternalOutput").ap()

    AF = mybir.ActivationFunctionType
    OP = mybir.AluOpType
    b6f = float(b6[0])

    with tile.TileContext(nc) as tc:
        with (
            tc.tile_pool(name="const", bufs=1) as cp,
            tc.tile_pool(name="us_p", bufs=4) as us_p,
            tc.tile_pool(name="mg_p", bufs=4) as mg_p,
            tc.tile_pool(name="h_p", bufs=2) as h_p,
            tc.tile_pool(name="pd_p", bufs=4, space="PSUM") as pd_p,
            tc.tile_pool(name="po_p", bufs=2, space="PSUM") as po_p,
            tc.tile_pool(name="pin_p", bufs=1, space="PSUM") as pin_p,
            tc.tile_pool(name="ph_p", bufs=1, space="PSUM") as ph_p,
        ):
            featT = cp.tile([16, T * BL], BF16, tag="featT")
            encsb = cp.tile([F, ENC_TOT * BL], BF16, tag="encsb")
            wb = cp.tile([128, _CB], BF16, tag="wb")
            bias = cp.tile([64, 1], F32, tag="bias")
            INS = [cp.tile([F, T * BL], BF16, tag=f"in{i}",
                           name=f"in{i}") for i in range(6)]
            sk = cp.tile([F, 6 * BL], BF16, tag="sk")
            yout_sb = cp.tile([1, T * BL], F32, tag="yout_sb")

            nc.vector.memset(bias[:], 0.0)
            if float(np.abs(b1).max()) != 0.0:
                for r0, v in enumerate(np.asarray(b1, np.float32)):
                    nc.vector.memset(bias[r0:r0 + 1, 0:1], float(v))

            nc.sync.dma_start(wb[:], wb_in[:])
            nc.sync.dma_start(featT[:], feat_in[:])
            nc.gpsimd.dma_start(encsb[:], enc_in[:])

            W2s = wb[0:64, _C_W2:_C_W2 + 128]
            W3s = wb[0:64, _C_W3:_C_W3 + 128]
            M34s = wb[0:64, _C_M34:_C_M34 + 128]
            W4s = wb[0:64, _C_W4:_C_W4 + 128]
            W5s = [wb[0:64, _C_W5 + j * 128:_C_W5 + (j + 1) * 128]
                   for j in range(6)]
            W1s = wb[0:16, _C_W1:_C_W1 + 64]
            As = wb[:, _C_A:_C_A + 64]
            W6s = wb[:, _C_W6:_C_W6 + 1]
            b1s = bias[:, 0:1]

            def blk(t):
                return slice(t * BL, (t + 1) * BL)

            def state_ap(i, t):
                d = DILATIONS[i]
                if t < d:
                    return encsb[:, blk(ENC_OFF[i] + t)]
                return INS[i + 1][:, blk(t - d)]

            # HAM warm-up: dense back-to-back matmuls flip the PE clock
            # gate to 8/8 (2.4 GHz); reads a memset scratch tile so it
            # runs concurrently with (not after) the input DMAs.
            wsrc = us_p.tile([128, 2 * BL], BF16, tag="us", name="wsrc")
            nc.vector.memset(wsrc[:], 0.25)
            wu = pd_p.tile([128, BL], F32, tag="pd", name="warmup")
            for w in range(40):
                nc.tensor.matmul(wu[:], wsrc[:, 0:128],
                                 wsrc[:, BL:BL + 128],
                                 start=(w == 0), stop=(w == 39))

            pin = pin_p.tile([64, BL], F32, tag="pin", name="pin0")
            nc.tensor.matmul(pin[:], W1s, featT[:, blk(0)],
                             start=True, stop=True)

            pds = {}
            for k in range(4):
                pds[k] = pd_p.tile([128, BL], F32, tag="pd",
                                   name=f"pd0_{k}")
                nc.tensor.matmul(pds[k][:], W2s, state_ap(k, 0),
                                 start=True, stop=False)

            for t in range(T):
                # -- step head: IN0 = tanh(pin).  The W2 births for this
                # step were issued at the previous tail (after A@h) so
                # they execute DURING this tanh and the W3 below runs
                # back-to-back behind them.
                nc.scalar.activation(INS[0][:, blk(t)], pin[:],
                                     AF.Tanh, bias=b1s)
                IN0 = INS[0][:, blk(t)]
                nc.tensor.matmul(pds[0][:], W3s, IN0,
                                 start=False, stop=True)
                nc.tensor.matmul(pds[1][:], W3s, IN0,
                                 start=False, stop=False)

                ph = ph_p.tile([HID, BL], F32, tag="ph", name=f"ph{t}")
                pos = [None] * 6
                pds_n = {}

                def blk_s(j):
                    return slice(j * BL, (j + 1) * BL)

                def relu_skip(j):
                    nc.scalar.activation(sk[:, blk_s(j)], pos[j][0:64, :],
                                         AF.Relu)

                for i in range(6):
                    # deferred relu (j = i-2) first: fills the ACT gap
                    # while pd_i's chain matmul lands.
                    if i >= 2:
                        relu_skip(i - 2)
                    us = us_p.tile([128, 2 * BL], BF16, tag="us")
                    nc.scalar.activation(us[:, 0:BL], pds[i][:], AF.Tanh)
                    uf = us[0:64, 0:BL]
                    # ready fillers -- execute DURING the tanh.
                    if 1 <= i < 5:
                        nc.tensor.matmul(pds[i + 1][:], W3s,
                                         INS[i - 1][:, blk(t)],
                                         start=False, stop=False)
                    if i == 0:
                        pds[4] = pd_p.tile([128, BL], F32, tag="pd",
                                           name=f"pd{t}_4")
                        nc.tensor.matmul(pds[4][:], W2s, state_ap(4, t),
                                         start=True, stop=False)
                    if i == 1:
                        pds[5] = pd_p.tile([128, BL], F32, tag="pd",
                                           name=f"pd{t}_5")
                        nc.tensor.matmul(pds[5][:], W2s, state_ap(5, t),
                                         start=True, stop=False)
                    if i >= 2:
                        j = i - 2
                        nc.tensor.matmul(ph[:], W5s[j], sk[:, blk_s(j)],
                                         start=(j == 0), stop=False)
                    # u_f-gated fillers (mg2 = u_f*u_g + u_f) -- start at
                    # tanh-data, cover the DVE window so the prod halves
                    # run back-to-back.
                    if i < 5:
                        nc.tensor.matmul(pds[i + 1][:], M34s, uf,
                                         start=False, stop=False)
                    po = po_p.tile([128, BL], F32, tag="po",
                                   name=f"po{t}_{i}")
                    nc.tensor.matmul(po[:], W4s, uf, start=True,
                                     stop=False)
                    pos[i] = po
                    if i == 5 and t + 1 < T:
                        for k in (0, 1):
                            pds_n[k] = pd_p.tile(
                                [128, BL], F32, tag="pd",
                                name=f"pd{t + 1}_{k}")
                            nc.tensor.matmul(pds_n[k][:], W2s,
                                             state_ap(k, t + 1),
                                             start=True, stop=False)
                    # DVE: copy u_g beside u_f (two-tensor DVE ops need
                    # equal base partitions), then the product.
                    nc.vector.tensor_copy(us[0:64, BL:2 * BL],
                                          us[64:128, 0:BL])
                    mg = mg_p.tile([64, BL], BF16, tag="mg")
                    nc.vector.tensor_mul(mg[:], us[0:64, BL:2 * BL], uf)
                    # prod-gated chain group.
                    if i < 5:
                        nc.tensor.matmul(pds[i + 1][:], M34s, mg[:],
                                         start=False, stop=True)
                    nc.tensor.matmul(po[:], W4s, mg[:], start=False,
                                     stop=True)
                    # off-chain telescope pair AFTER the chain group: its
                    # ready half bypass-fills the last pre-chain gap.
                    if i + 2 <= 5:
                        nc.tensor.matmul(pds[i + 2][:], M34s, uf,
                                         start=False, stop=False)
                        nc.tensor.matmul(pds[i + 2][:], M34s, mg[:],
                                         start=False, stop=False)
                    # deferred residual add (j = i-1) off the chain.
                    if i >= 1:
                        j = i - 1
                        nc.vector.tensor_add(INS[j + 1][:, blk(t)],
                                             pos[j][64:128, :],
                                             INS[j][:, blk(t)])

                # -- tail
                relu_skip(4)
                nc.vector.tensor_scalar_max(sk[:, blk_s(5)],
                                            pos[5][0:64, :], 0.0)
                nc.tensor.matmul(ph[:], W5s[4], sk[:, blk_s(4)],
                                 start=False, stop=False)
                nc.tensor.matmul(ph[:], W5s[5], sk[:, blk_s(5)],
                                 start=False, stop=True)
                h = h_p.tile([HID, BL], BF16, tag="h")
                nc.vector.tensor_scalar_max(h[:], ph[:], 0.0)
                if t + 1 < T:
                    pin = pin_p.tile([64, BL], F32, tag="pin",
                                     name=f"pin{t + 1}")
                    nc.tensor.matmul(pin[:], W1s, featT[:, blk(t + 1)],
                                     start=True, stop=False)
                    nc.tensor.matmul(pin[:], As, h[:],
                                     start=False, stop=True)
                py = po_p.tile([1, BL], F32, tag="po", name=f"py{t}")
                nc.tensor.matmul(py[:], W6s, h[:], start=True, stop=True)
                nc.vector.tensor_scalar_add(yout_sb[0:1, blk(t)], py[:],
                                            b6f)
                if t % 6 == 5 or t == T - 2:
                    c0 = (t // 6) * 6 * BL
                    c1 = (t + 1) * BL
                    nc.sync.dma_start(y_out[0:1, c0:c1],
                                      yout_sb[0:1, c0:c1])
                if t + 1 < T:
                    for k in (2, 3):
                        pds_n[k] = pd_p.tile([128, BL], F32, tag="pd",
                                             name=f"pd{t + 1}_{k}")
                        nc.tensor.matmul(pds_n[k][:], W2s,
                                         state_ap(k, t + 1),
                                         start=True, stop=False)
                    pds = pds_n


    nc.compile()
    return nc


def _pack_inputs(decoder_features, decoder_init_input, encoder_states,
                 W1, W2, W3, W4, W5, W6):
    """Host-side shard + transpose + bf16-cast.  Returns per-core in_maps."""
    wbp = np.zeros((128, _CB), np.float32)
    wbp[0:64, _C_W2:_C_W2 + 64] = W2[:, 0:64]
    wbp[0:64, _C_W2 + 64:_C_W2 + 128] = 0.5 * W2[:, 64:128]
    wbp[0:64, _C_W3:_C_W3 + 64] = W3[:, 0:64]
    wbp[0:64, _C_W3 + 64:_C_W3 + 128] = 0.5 * W3[:, 64:128]
    M34 = 0.5 * (W4[:, 64:128] @ W3)                 # [64, 128]
    wbp[0:64, _C_M34:_C_M34 + 64] = M34[:, 0:64]
    wbp[0:64, _C_M34 + 64:_C_M34 + 128] = 0.5 * M34[:, 64:128]
    wbp[0:64, _C_W4:_C_W4 + 128] = 0.5 * W4
    for j in range(6):
        wbp[0:64, _C_W5 + j * 128:_C_W5 + (j + 1) * 128] = \
            W5[j * 64:(j + 1) * 64, :]
    wbp[0:16, _C_W1:_C_W1 + 64] = W1
    wbp[:, _C_A:_C_A + 64] = W6 @ W1[0:1, :]         # [128, 64]
    wbp[:, _C_W6:_C_W6 + 1] = W6
    wb_bits = _bf16(wbp)

    in_maps = []
    for c in range(N_CORES):
        s = slice(c * BL, (c + 1) * BL)
        # featT [16, T*BL]: row 0 blk 0 = init, zero elsewhere;
        # rows 1:16 = features^T.
        ft = np.zeros((16, T, BL), np.float32)
        ft[0, 0, :] = decoder_init_input[s, 0]
        ft[1:16] = decoder_features[s].transpose(2, 1, 0)
        # encpack [64, ENC_TOT*BL]
        ep = np.zeros((F, ENC_TOT, BL), np.float32)
        for i, d in enumerate(DILATIONS):
            n = ENC_N[i]
            ep[:, ENC_OFF[i]:ENC_OFF[i] + n, :] = \
                encoder_states[i, s, L - d:L - d + n, :].transpose(2, 1, 0)
        in_maps.append({
            "featsrc": _bf16(ft.reshape(16, T * BL)),
            "encpack": _bf16(ep.reshape(F, ENC_TOT * BL)),
            "wpackb": wb_bits,
        })
    return in_maps


def kernel(**inputs):
    decoder_features = np.asarray(inputs["decoder_features"], np.float32)
    decoder_init_input = np.asarray(inputs["decoder_init_input"], np.float32)
    encoder_states = np.asarray(inputs["encoder_states"], np.float32)
    W1 = np.asarray(inputs["W1"], np.float32)
    b1 = np.asarray(inputs["b1"], np.float32)
    W2 = np.asarray(inputs["W2"], np.float32)
    b2 = np.asarray(inputs["b2"], np.float32)
    W3 = np.asarray(inputs["W3"], np.float32)
    W4 = np.asarray(inputs["W4"], np.float32)
    b4 = np.asarray(inputs["b4"], np.float32)
    W5 = np.asarray(inputs["W5"], np.float32)
    b5 = np.asarray(inputs["b5"], np.float32)
    W6 = np.asarray(inputs["W6"], np.float32)
    b6 = np.asarray(inputs["b6"], np.float32)
    assert float(np.abs(b2).max()) == 0.0, "kernel assumes b2 == 0"
    assert float(np.abs(b4).max()) == 0.0, \
        "kernel's telescoped dilated accumulation assumes b4 == 0"
    assert float(np.abs(b5).max()) == 0.0, "kernel assumes b5 == 0"

    key = "nc"
    if key not in _CACHE:
        _CACHE[key] = _build(b1, b6)
    nc = _CACHE[key]

    in_maps = _pack_inputs(decoder_features, decoder_init_input,
                           encoder_states, W1, W2, W3, W4, W5, W6)
    res = run_bass_kernel_spmd(nc, in_maps, list(range(N_CORES)))

    out = np.empty((B, T, 1), np.float32)
    for c in range(N_CORES):
        y = res.results[c]["yout"].reshape(T, BL)
        out[c * BL:(c + 1) * BL, :, 0] = y.T
    return out


# revision 10
# speedup vs baseline: 1.0925x; 1.0019x over previous
"""WaveNet-style decoder (nn_DecoderV2) on 8 TRN2 NeuronCores.

Strategy: pure data parallel over batch (1024 -> 8 x 128). Per core the
recurrence runs with activations stored transposed [feature, batch] so the
batch lives on the free dim and every dense layer is a TensorE matmul with
stationary weights.

Layout (f|g on the PARTITION dim -- single matmuls everywhere):
  - pd [128, BL] PSUM: rows 0:64 = conv filter preactivation, rows 64:128
    = 0.5*gate preactivation, so each dense is ONE K=64, M=128 matmul
    (W2/W3/M34 packed as [64, 128] = [f | 0.5*g]) and the nonlinearity is
    ONE [128, BL] ACT u = tanh(pd) (sigmoid(g) = (tanh(g/2)+1)/2).
  - Gate: two-tensor DVE ops need equal base partitions, so u_g is copied
    beside u_f (DVE tensor_copy with output partition shift) and ONE
    same-base mul gives prod = u_f*u_g.  mg2 = 2*gated = u_f + prod is
    never materialized: every consumer (chain M34, po=W4', off-chain M34)
    is a PAIR of matmuls @u_f and @prod.  The @u_f halves are gated only
    on the tanh, so they start ~300 ns early and keep the PE warm; the
    @prod halves then run back-to-back right as the DVE mul lands.
  - States are read IN PLACE: layer i's state at step t is IN[i+1] block
    (t-d) for t >= d (the residual outputs ARE the appended state) or the
    pre-packed encoder slice for t < d.  Zero state-copy DMAs.
  - Depth-2 telescope: pd_k = W2@state_k + W3@IN_{k-2} + M34@(mg2_{k-2}
    + mg2_{k-1}); the deferred W3 (issued as a ready filler during the
    next tanh) and the off-chain M34 pair keep only one matmul pair on
    the chain per layer.  Residual adds (DVE) are deferred off-chain.
  - Feedback tail is folded: pin(t+1) = W1@featT(t+1) + (W6 (x) W1[0])@h,
    a rank-1 composed weight A, skipping y -> featT -> W1 on the chain
    (featT row 0 is zero for t >= 1 and holds decoder_init at t = 0, so
    one K=16 W1 matmul serves both the t=0 pin and the static part).
  - Skips: sk [64, 6*BL]; relu on ACT deferred by 2 layers (fills the
    ACT gap between tanhs); layer 5's relu on DVE at the tail; W5 as six
    K=64 matmuls accumulating in one PSUM bank.
  - PSUM: 4 pd banks (rotating; pd_4/pd_5 born after u_0/u_1 free their
    bank, pd_0..3 of step t+1 born in step t's tail as PE fillers), 2 po,
    1 pin, 1 ph = 8 banks.
  - PE warm-up on a memset scratch tile (no DMA dependency); yout DMA'd
    out in chunks as steps complete.

All matmul operands bf16 (PSUM fp32); b2/b4/b5 asserted zero (b1, b6
honored via ACT bias / DVE scalar).  HW exec ~256 us vs 306 us baseline;
the remaining span is chain latency: per layer tanh ~440 ns + DVE copy
+ mul ~480 ns + chain matmul ~115 ns + semaphores ~190 ns.
"""

import numpy as np

import concourse.bacc as bacc
import concourse.mybir as mybir
import concourse.tile as tile
from concourse.bass_utils import run_bass_kernel_spmd

F32 = mybir.dt.float32
BF16 = mybir.dt.bfloat16

N_CORES = 8
B = 1024
BL = B // N_CORES          # 128 batch per core
T = 24
F = 64
HID = 128
DILATIONS = (1, 2, 4, 8, 16, 32)
L = 168
ENC_N = [min(d, T) for d in DILATIONS]
ENC_OFF = np.concatenate([[0], np.cumsum(ENC_N)]).astype(int).tolist()
ENC_TOT = int(np.sum(ENC_N))                     # 55

# wpack (bf16) column layout; [f | 0.5g] pairs live on cols, K on rows.
_C_W2 = 0          # [64, 128]
_C_W3 = 128        # [64, 128]
_C_M34 = 256       # [64, 128]  0.5*(W4r@W3) with g-half halved again
_C_W4 = 384        # [64, 128]  0.5*W4
_C_W5 = 512        # 6 x [64, 128] chunks
_C_W1 = 1280       # [16, 64]
_C_A = 1344        # [128, 64]  W6 (x) W1[0]
_C_W6 = 1408       # [128, 1]
_CB = 1409

_CACHE = {}


def _bf16(a):
    """fp32 array -> ml_dtypes.bfloat16 (round to nearest even)."""
    import ml_dtypes
    return np.asarray(a, dtype=np.float32).astype(ml_dtypes.bfloat16)


def _build(b1, b6):
    nc = bacc.Bacc("TRN2", target_bir_lowering=False, debug=False,
                   num_devices=N_CORES)

    feat_in = nc.dram_tensor("featsrc", [16, T * BL], BF16,
                             kind="ExternalInput").ap()
    enc_in = nc.dram_tensor("encpack", [F, ENC_TOT * BL], BF16,
                            kind="ExternalInput").ap()
    wb_in = nc.dram_tensor("wpackb", [128, _CB], BF16,
                           kind="ExternalInput").ap()
    y_out = nc.dram_tensor("yout", [1, T * BL], F32,
                           kind="ExternalOutput").ap()

    AF = mybir.ActivationFunctionType
    OP = mybir.AluOpType
    b6f = float(b6[0])

    with tile.TileContext(nc) as tc:
        with (
            tc.tile_pool(name="const", bufs=1) as cp,
            tc.tile_pool(name="us_p", bufs=4) as us_p,
            tc.tile_pool(name="mg_p", bufs=4) as mg_p,
            tc.tile_pool(name="h_p", bufs=2) as h_p,
            tc.tile_pool(name="pd_p", bufs=4, space="PSUM") as pd_p,
            tc.tile_pool(name="po_p", bufs=2, space="PSUM") as po_p,
            tc.tile_pool(name="pin_p", bufs=1, space="PSUM") as pin_p,
            tc.tile_pool(name="ph_p", bufs=1, space="PSUM") as ph_p,
        ):
            featT = cp.tile([16, T * BL], BF16, tag="featT")
            encsb = cp.tile([F, ENC_TOT * BL], BF16, tag="encsb")
            wb = cp.tile([128, _CB], BF16, tag="wb")
            bias = cp.tile([64, 1], F32, tag="bias")
            INS = [cp.tile([F, T * BL], BF16, tag=f"in{i}",
                           name=f"in{i}") for i in range(6)]
            sk = cp.tile([F, 6 * BL], BF16, tag="sk")
            yout_sb = cp.tile([1, T * BL], F32, tag="yout_sb")

            nc.vector.memset(bias[:], 0.0)
            if float(np.abs(b1).max()) != 0.0:
                for r0, v in enumerate(np.asarray(b1, np.float32)):
                    nc.vector.memset(bias[r0:r0 + 1, 0:1], float(v))

            nc.sync.dma_start(wb[:], wb_in[:])
            nc.sync.dma_start(featT[:], feat_in[:])
            nc.gpsimd.dma_start(encsb[:], enc_in[:])

            W2s = wb[0:64, _C_W2:_C_W2 + 128]
            W3s = wb[0:64, _C_W3:_C_W3 + 128]
            M34s = wb[0:64, _C_M34:_C_M34 + 128]
            W4s = wb[0:64, _C_W4:_C_W4 + 128]
            W5s = [wb[0:64, _C_W5 + j * 128:_C_W5 + (j + 1) * 128]
                   for j in range(6)]
            W1s = wb[0:16, _C_W1:_C_W1 + 64]
            As = wb[:, _C_A:_C_A + 64]
            W6s = wb[:, _C_W6:_C_W6 + 1]
            b1s = bias[:, 0:1]

            def blk(t):
                return slice(t * BL, (t + 1) * BL)

            def state_ap(i, t):
                d = DILATIONS[i]
                if t < d:
                    return encsb[:, blk(ENC_OFF[i] + t)]
                return INS[i + 1][:, blk(t - d)]

            # HAM warm-up: dense back-to-back matmuls flip the PE clock
            # gate to 8/8 (2.4 GHz); reads a memset scratch tile so it
            # runs concurrently with (not after) the input DMAs.
            wsrc = us_p.tile([128, 2 * BL], BF16, tag="us", name="wsrc")
            nc.vector.memset(wsrc[:], 0.25)
            wu = pd_p.tile([128, BL], F32, tag="pd", name="warmup")
            for w in range(40):
                nc.tensor.matmul(wu[:], wsrc[:, 0:128],
                                 wsrc[:, BL:BL + 128],
                                 start=(w == 0), stop=(w == 39))

            pin = pin_p.tile([64, BL], F32, tag="pin", name="pin0")
            nc.tensor.matmul(pin[:], W1s, featT[:, blk(0)],
                             start=True, stop=True)

            pds = {}
            for k in range(4):
                pds[k] = pd_p.tile([128, BL], F32, tag="pd",
                                   name=f"pd0_{k}")
                nc.tensor.matmul(pds[k][:], W2s, state_ap(k, 0),
                                 start=True, stop=False)

            for t in range(T):
                # -- step head: IN0 = tanh(pin).  The W2 births for this
                # step were issued at the previous tail (after A@h) so
                # they execute DURING this tanh and the W3 below runs
                # back-to-back behind them.
                nc.scalar.activation(INS[0][:, blk(t)], pin[:],
                                     AF.Tanh, bias=b1s)
                IN0 = INS[0][:, blk(t)]
                nc.tensor.matmul(pds[0][:], W3s, IN0,
                                 start=False, stop=True)
                nc.tensor.matmul(pds[1][:], W3s, IN0,
                                 start=False, stop=False)

                ph = ph_p.tile([HID, BL], F32, tag="ph", name=f"ph{t}")
                pos = [None] * 6
                pds_n = {}

                def blk_s(j):
                    return slice(j * BL, (j + 1) * BL)

                def relu_skip(j):
                    nc.scalar.activation(sk[:, blk_s(j)], pos[j][0:64, :],
                                         AF.Relu)

                for i in range(6):
                    # deferred relu (j = i-2) first: fills the ACT gap
                    # while pd_i's chain matmul lands.
                    if i >= 2:
                        relu_skip(i - 2)
                    us = us_p.tile([128, 2 * BL], BF16, tag="us")
                    nc.scalar.activation(us[:, 0:BL], pds[i][:], AF.Tanh)
                    uf = us[0:64, 0:BL]
                    # ready fillers -- execute DURING the tanh.
                    if 1 <= i < 5:
                        nc.tensor.matmul(pds[i + 1][:], W3s,
                                         INS[i - 1][:, blk(t)],
                                         start=False, stop=False)
                    if i == 0:
                        pds[4] = pd_p.tile([128, BL], F32, tag="pd",
                                           name=f"pd{t}_4")
                        nc.tensor.matmul(pds[4][:], W2s, state_ap(4, t),
                                         start=True, stop=False)
                    if i == 1:
                        pds[5] = pd_p.tile([128, BL], F32, tag="pd",
                                           name=f"pd{t}_5")
                        nc.tensor.matmul(pds[5][:], W2s, state_ap(5, t),
                                         start=True, stop=False)
                    if i >= 2:
                        j = i - 2
                        nc.tensor.matmul(ph[:], W5s[j], sk[:, blk_s(j)],
                                         start=(j == 0), stop=False)
                    # u_f-gated fillers (mg2 = u_f*u_g + u_f) -- start at
                    # tanh-data, cover the DVE window so the prod halves
                    # run back-to-back.
                    if i < 5:
                        nc.tensor.matmul(pds[i + 1][:], M34s, uf,
                                         start=False, stop=False)
                    po = po_p.tile([128, BL], F32, tag="po",
                                   name=f"po{t}_{i}")
                    nc.tensor.matmul(po[:], W4s, uf, start=True,
                                     stop=False)
                    pos[i] = po
                    if i == 5 and t + 1 < T:
                        for k in (0, 1):
                            pds_n[k] = pd_p.tile(
                                [128, BL], F32, tag="pd",
                                name=f"pd{t + 1}_{k}")
                            nc.tensor.matmul(pds_n[k][:], W2s,
                                             state_ap(k, t + 1),
                                             start=True, stop=False)
                    # DVE: copy u_g beside u_f (two-tensor DVE ops need
                    # equal base partitions), then the product.
                    nc.vector.tensor_copy(us[0:64, BL:2 * BL],
                                          us[64:128, 0:BL])
                    mg = mg_p.tile([64, BL], BF16, tag="mg")
                    nc.vector.tensor_mul(mg[:], us[0:64, BL:2 * BL], uf)
                    # prod-gated chain group.
                    if i < 5:
                        nc.tensor.matmul(pds[i + 1][:], M34s, mg[:],
                                         start=False, stop=True)
                    nc.tensor.matmul(po[:], W4s, mg[:], start=False,
                                     stop=True)
                    # off-chain telescope pair AFTER the chain group: its
                    # ready half bypass-fills the last pre-chain gap.
                    if i + 2 <= 5:
                        nc.tensor.matmul(pds[i + 2][:], M34s, uf,
                                         start=False, stop=False)
                        nc.tensor.matmul(pds[i + 2][:], M34s, mg[:],
                                         start=False, stop=False)
                    # deferred residual add (j = i-1) off the chain.
                    if i >= 1:
                        j = i - 1
                        nc.vector.tensor_add(INS[j + 1][:, blk(t)],
                                             pos[j][64:128, :],
                                             INS[j][:, blk(t)])

                # -- tail
                relu_skip(4)
                nc.vector.tensor_scalar_max(sk[:, blk_s(5)],
                                            pos[5][0:64, :], 0.0)
                nc.tensor.matmul(ph[:], W5s[4], sk[:, blk_s(4)],
                                 start=False, stop=False)
                nc.tensor.matmul(ph[:], W5s[5], sk[:, blk_s(5)],
                                 start=False, stop=True)
                h = h_p.tile([HID, BL], BF16, tag="h")
                nc.vector.tensor_scalar_max(h[:], ph[:], 0.0)
                if t + 1 < T:
                    pin = pin_p.tile([64, BL], F32, tag="pin",
                                     name=f"pin{t + 1}")
                    nc.tensor.matmul(pin[:], W1s, featT[:, blk(t + 1)],
                                     start=True, stop=False)
                    nc.tensor.matmul(pin[:], As, h[:],
                                     start=False, stop=True)
                py = po_p.tile([1, BL], F32, tag="po", name=f"py{t}")
                nc.tensor.matmul(py[:], W6s, h[:], start=True, stop=True)
                nc.vector.tensor_scalar_add(yout_sb[0:1, blk(t)], py[:],
                                            b6f)
                if t % 6 == 5 or t == T - 2:
                    c0 = (t // 6) * 6 * BL
                    c1 = (t + 1) * BL
                    nc.sync.dma_start(y_out[0:1, c0:c1],
                                      yout_sb[0:1, c0:c1])
                if t + 1 < T:
                    for k in (2, 3):
                        pds_n[k] = pd_p.tile([128, BL], F32, tag="pd",
                                             name=f"pd{t + 1}_{k}")
                        nc.tensor.matmul(pds_n[k][:], W2s,
                                         state_ap(k, t + 1),
                                         start=True, stop=False)
                    pds = pds_n


    nc.compile()
    return nc


def _pack_inputs(decoder_features, decoder_init_input, encoder_states,
                 W1, W2, W3, W4, W5, W6):
    """Host-side shard + transpose + bf16-cast.  Returns per-core in_maps."""
    wbp = np.zeros((128, _CB), np.float32)
    wbp[0:64, _C_W2:_C_W2 + 64] = W2[:, 0:64]
    wbp[0:64, _C_W2 + 64:_C_W2 + 128] = 0.5 * W2[:, 64:128]
    wbp[0:64, _C_W3:_C_W3 + 64] = W3[:, 0:64]
    wbp[0:64, _C_W3 + 64:_C_W3 + 128] = 0.5 * W3[:, 64:128]
    M34 = 0.5 * (W4[:, 64:128] @ W3)                 # [64, 128]
    wbp[0:64, _C_M34:_C_M34 + 64] = M34[:, 0:64]
    wbp[0:64, _C_M34 + 64:_C_M34 + 128] = 0.5 * M34[:, 64:128]
    wbp[0:64, _C_W4:_C_W4 + 128] = 0.5 * W4
    for j in range(6):
        wbp[0:64, _C_W5 + j * 128:_C_W5 + (j + 1) * 128] = \
            W5[j * 64:(j + 1) * 64, :]
    wbp[0:16, _C_W1:_C_W1 + 64] = W1
    wbp[:, _C_A:_C_A + 64] = W6 @ W1[0:1, :]         # [128, 64]
    wbp[:, _C_W6:_C_W6 + 1] = W6
    wb_bits = _bf16(wbp)

    in_maps = []
    for c in range(N_CORES):
        s = slice(c * BL, (c + 1) * BL)
        # featT [16, T*BL]: row 0 blk 0 = init, zero elsewhere;
        # rows 1:16 = features^T.
        ft = np.zeros((16, T, BL), np.float32)
        ft[0, 0, :] = decoder_init_input[s, 0]
        ft[1:16] = decoder_features[s].transpose(2, 1, 0)
        # encpack [64, ENC_TOT*BL]
        ep = np.zeros((F, ENC_TOT, BL), np.float32)
        for i, d in enumerate(DILATIONS):
            n = ENC_N[i]
            ep[:, ENC_OFF[i]:ENC_OFF[i] + n, :] = \
                encoder_states[i, s, L - d:L - d + n, :].transpose(2, 1, 0)
        in_maps.append({
            "featsrc": _bf16(ft.reshape(16, T * BL)),
            "encpack": _bf16(ep.reshape(F, ENC_TOT * BL)),
            "wpackb": wb_bits,
        })
    return in_maps


def kernel(**inputs):
    decoder_features = np.asarray(inputs["decoder_features"], np.float32)
    decoder_init_input = np.asarray(inputs["decoder_init_input"], np.float32)
    encoder_states = np.asarray(inputs["encoder_states"], np.float32)
    W1 = np.asarray(inputs["W1"], np.float32)
    b1 = np.asarray(inputs["b1"], np.float32)
    W2 = np.asarray(inputs["W2"], np.float32)
    b2 = np.asarray(inputs["b2"], np.float32)
    W3 = np.asarray(inputs["W3"], np.float32)
    W4 = np.asarray(inputs["W4"], np.float32)
    b4 = np.asarray(inputs["b4"], np.float32)
    W5 = np.asarray(inputs["W5"], np.float32)
    b5 = np.asarray(inputs["b5"], np.float32)
    W6 = np.asarray(inputs["W6"], np.float32)
    b6 = np.asarray(inputs["b6"], np.float32)
    assert float(np.abs(b2).max()) == 0.0, "kernel assumes b2 == 0"
    assert float(np.abs(b4).max()) == 0.0, \
        "kernel's telescoped dilated accumulation assumes b4 == 0"
    assert float(np.abs(b5).max()) == 0.0, "kernel assumes b5 == 0"

    key = "nc"
    if key not in _CACHE:
        _CACHE[key] = _build(b1, b6)
    nc = _CACHE[key]

    in_maps = _pack_inputs(decoder_features, decoder_init_input,
                           encoder_states, W1, W2, W3, W4, W5, W6)
    res = run_bass_kernel_spmd(nc, in_maps, list(range(N_CORES)))

    out = np.empty((B, T, 1), np.float32)
    for c in range(N_CORES):
        y = res.results[c]["yout"].reshape(T, BL)
        out[c * BL:(c + 1) * BL, :, 0] = y.T
    return out


# revision 11
# speedup vs baseline: 1.0979x; 1.0050x over previous
"""WaveNet-style decoder (nn_DecoderV2) on 8 TRN2 NeuronCores.

Strategy: pure data parallel over batch (1024 -> 8 x 128). Per core the
recurrence runs with activations stored transposed [feature, batch] so the
batch lives on the free dim and every dense layer is a TensorE matmul with
stationary weights.

Layout (f|g on the PARTITION dim -- single matmuls everywhere):
  - pd [128, BL] PSUM: rows 0:64 = conv filter preactivation, rows 64:128
    = 0.5*gate preactivation, so each dense is ONE K=64, M=128 matmul
    (W2/W3/M34 packed as [64, 128] = [f | 0.5*g]) and the nonlinearity is
    ONE [128, BL] ACT u = tanh(pd) (sigmoid(g) = (tanh(g/2)+1)/2).
  - Gate: two-tensor DVE ops need equal base partitions, so u_g is copied
    beside u_f (DVE tensor_copy with output partition shift) and ONE
    same-base mul gives prod = u_f*u_g.  mg2 = 2*gated = u_f + prod is
    never materialized: every consumer (chain M34, po=W4', off-chain M34)
    is a PAIR of matmuls @u_f and @prod.  The @u_f halves are gated only
    on the tanh, so they start ~300 ns early and keep the PE warm; the
    @prod halves then run back-to-back right as the DVE mul lands.
  - States are read IN PLACE: layer i's state at step t is IN[i+1] block
    (t-d) for t >= d (the residual outputs ARE the appended state) or the
    pre-packed encoder slice for t < d.  Zero state-copy DMAs.
  - Depth-2 telescope: pd_k = W2@state_k + W3@IN_{k-2} + M34@(mg2_{k-2}
    + mg2_{k-1}); the deferred W3 (issued as a ready filler during the
    next tanh) and the off-chain M34 pair keep only one matmul pair on
    the chain per layer.  Residual adds (DVE) are deferred off-chain.
  - Feedback tail is folded: pin(t+1) = W1@featT(t+1) + (W6 (x) W1[0])@h,
    a rank-1 composed weight A, skipping y -> featT -> W1 on the chain
    (featT row 0 is zero for t >= 1 and holds decoder_init at t = 0, so
    one K=16 W1 matmul serves both the t=0 pin and the static part).
  - Skips: sk [64, 6*BL]; relu on ACT deferred by 2 layers (fills the
    ACT gap between tanhs); layer 5's relu on DVE at the tail; W5 as six
    K=64 matmuls accumulating in one PSUM bank.
  - PSUM: 4 pd banks (rotating; pd_4/pd_5 born after u_0/u_1 free their
    bank, pd_0..3 of step t+1 born in step t's tail as PE fillers), 2 po,
    1 pin, 1 ph = 8 banks.
  - PE warm-up on a memset scratch tile (no DMA dependency); yout DMA'd
    out in chunks as steps complete.

All matmul operands bf16 (PSUM fp32); b2/b4/b5 asserted zero (b1, b6
honored via ACT bias / DVE scalar).  HW exec ~256 us vs 306 us baseline;
the remaining span is chain latency: per layer tanh ~440 ns + DVE copy
+ mul ~480 ns + chain matmul ~115 ns + semaphores ~190 ns.
"""

import numpy as np

import concourse.bacc as bacc
import concourse.mybir as mybir
import concourse.tile as tile
from concourse.bass_utils import run_bass_kernel_spmd

F32 = mybir.dt.float32
BF16 = mybir.dt.bfloat16

N_CORES = 8
B = 1024
BL = B // N_CORES          # 128 batch per core
T = 24
F = 64
HID = 128
DILATIONS = (1, 2, 4, 8, 16, 32)
L = 168
ENC_N = [min(d, T) for d in DILATIONS]
ENC_OFF = np.concatenate([[0], np.cumsum(ENC_N)]).astype(int).tolist()
ENC_TOT = int(np.sum(ENC_N))                     # 55

# wpack (bf16) column layout; [f | 0.5g] pairs live on cols, K on rows.
_C_W2 = 0          # [64, 128]
_C_W3 = 128        # [64, 128]
_C_M34 = 256       # [64, 128]  0.5*(W4r@W3) with g-half halved again
_C_W4 = 384        # [64, 128]  0.5*W4
_C_W5 = 512        # 6 x [64, 128] chunks
_C_W1 = 1280       # [16, 64]
_C_A = 1344        # [128, 64]  W6 (x) W1[0]
_C_W6 = 1408       # [128, 1]
_CB = 1409

_CACHE = {}


def _bf16(a):
    """fp32 array -> ml_dtypes.bfloat16 (round to nearest even)."""
    import ml_dtypes
    return np.asarray(a, dtype=np.float32).astype(ml_dtypes.bfloat16)


def _build(b1, b6):
    nc = bacc.Bacc("TRN2", target_bir_lowering=False, debug=False,
                   num_devices=N_CORES)

    feat_in = nc.dram_tensor("featsrc", [16, T * BL], BF16,
                             kind="ExternalInput").ap()
    enc_in = nc.dram_tensor("encpack", [F, ENC_TOT * BL], BF16,
                            kind="ExternalInput").ap()
    wb_in = nc.dram_tensor("wpackb", [128, _CB], BF16,
                           kind="ExternalInput").ap()
    y_out = nc.dram_tensor("yout", [1, T * BL], F32,
                           kind="ExternalOutput").ap()

    AF = mybir.ActivationFunctionType
    OP = mybir.AluOpType
    b6f = float(b6[0])

    with tile.TileContext(nc) as tc:
        with (
            tc.tile_pool(name="const", bufs=1) as cp,
            tc.tile_pool(name="us_p", bufs=4) as us_p,
            tc.tile_pool(name="mg_p", bufs=4) as mg_p,
            tc.tile_pool(name="h_p", bufs=2) as h_p,
            tc.tile_pool(name="pd_p", bufs=4, space="PSUM") as pd_p,
            tc.tile_pool(name="po_p", bufs=2, space="PSUM") as po_p,
            tc.tile_pool(name="pin_p", bufs=1, space="PSUM") as pin_p,
            tc.tile_pool(name="ph_p", bufs=1, space="PSUM") as ph_p,
        ):
            featT = cp.tile([16, T * BL], BF16, tag="featT")
            encsb = cp.tile([F, ENC_TOT * BL], BF16, tag="encsb")
            wb = cp.tile([128, _CB], BF16, tag="wb")
            bias = cp.tile([64, 1], F32, tag="bias")
            INS = [cp.tile([F, T * BL], BF16, tag=f"in{i}",
                           name=f"in{i}") for i in range(6)]
            sk = cp.tile([F, 6 * BL], BF16, tag="sk")
            yout_sb = cp.tile([1, T * BL], F32, tag="yout_sb")

            nc.vector.memset(bias[:], 0.0)
            if float(np.abs(b1).max()) != 0.0:
                for r0, v in enumerate(np.asarray(b1, np.float32)):
                    nc.vector.memset(bias[r0:r0 + 1, 0:1], float(v))

            nc.sync.dma_start(wb[:], wb_in[:])
            nc.sync.dma_start(featT[:], feat_in[:])
            nc.gpsimd.dma_start(encsb[:], enc_in[:])

            W2s = wb[0:64, _C_W2:_C_W2 + 128]
            W3s = wb[0:64, _C_W3:_C_W3 + 128]
            M34s = wb[0:64, _C_M34:_C_M34 + 128]
            W4s = wb[0:64, _C_W4:_C_W4 + 128]
            W5s = [wb[0:64, _C_W5 + j * 128:_C_W5 + (j + 1) * 128]
                   for j in range(6)]
            W1s = wb[0:16, _C_W1:_C_W1 + 64]
            As = wb[:, _C_A:_C_A + 64]
            W6s = wb[:, _C_W6:_C_W6 + 1]
            b1s = bias[:, 0:1]

            def blk(t):
                return slice(t * BL, (t + 1) * BL)

            def state_ap(i, t):
                d = DILATIONS[i]
                if t < d:
                    return encsb[:, blk(ENC_OFF[i] + t)]
                return INS[i + 1][:, blk(t - d)]

            # HAM warm-up: dense back-to-back matmuls flip the PE clock
            # gate to 8/8 (2.4 GHz); reads a memset scratch tile so it
            # runs concurrently with (not after) the input DMAs.
            wsrc = us_p.tile([128, 2 * BL], BF16, tag="us", name="wsrc")
            nc.vector.memset(wsrc[:], 0.25)
            wu = pd_p.tile([128, BL], F32, tag="pd", name="warmup")
            for w in range(40):
                nc.tensor.matmul(wu[:], wsrc[:, 0:128],
                                 wsrc[:, BL:BL + 128],
                                 start=(w == 0), stop=(w == 39))

            pin = pin_p.tile([64, BL], F32, tag="pin", name="pin0")
            nc.tensor.matmul(pin[:], W1s, featT[:, blk(0)],
                             start=True, stop=True)

            pds = {}
            for k in range(4):
                pds[k] = pd_p.tile([128, BL], F32, tag="pd",
                                   name=f"pd0_{k}")
                nc.tensor.matmul(pds[k][:], W2s, state_ap(k, 0),
                                 start=True, stop=False)

            for t in range(T):
                # -- step head: IN0 = tanh(pin).  The W2 births for this
                # step were issued at the previous tail (after A@h) so
                # they execute DURING this tanh and the W3 below runs
                # back-to-back behind them.
                nc.scalar.activation(INS[0][:, blk(t)], pin[:],
                                     AF.Tanh, bias=b1s)
                IN0 = INS[0][:, blk(t)]
                nc.tensor.matmul(pds[0][:], W3s, IN0,
                                 start=False, stop=True)
                nc.tensor.matmul(pds[1][:], W3s, IN0,
                                 start=False, stop=False)

                ph = ph_p.tile([HID, BL], F32, tag="ph", name=f"ph{t}")
                pos = [None] * 6
                pds_n = {}

                def blk_s(j):
                    return slice(j * BL, (j + 1) * BL)

                def relu_skip(j):
                    nc.scalar.activation(sk[:, blk_s(j)], pos[j][0:64, :],
                                         AF.Relu)

                for i in range(6):
                    # deferred relu (j = i-2) first: fills the ACT gap
                    # while pd_i's chain matmul lands.
                    if i >= 2:
                        relu_skip(i - 2)
                    us = us_p.tile([128, 2 * BL], BF16, tag="us")
                    nc.scalar.activation(us[:, 0:BL], pds[i][:], AF.Tanh)
                    uf = us[0:64, 0:BL]
                    # ready fillers -- execute DURING the tanh.
                    if 1 <= i < 5:
                        nc.tensor.matmul(pds[i + 1][:], W3s,
                                         INS[i - 1][:, blk(t)],
                                         start=False, stop=False)
                    if i == 0:
                        pds[4] = pd_p.tile([128, BL], F32, tag="pd",
                                           name=f"pd{t}_4")
                        nc.tensor.matmul(pds[4][:], W2s, state_ap(4, t),
                                         start=True, stop=False)
                    if i == 1:
                        pds[5] = pd_p.tile([128, BL], F32, tag="pd",
                                           name=f"pd{t}_5")
                        nc.tensor.matmul(pds[5][:], W2s, state_ap(5, t),
                                         start=True, stop=False)
                    if i >= 2:
                        j = i - 2
                        nc.tensor.matmul(ph[:], W5s[j], sk[:, blk_s(j)],
                                         start=(j == 0), stop=False)
                    # u_f-gated fillers (mg2 = u_f*u_g + u_f) -- start at
                    # tanh-data, cover the DVE window so the prod halves
                    # run back-to-back.
                    if i < 5:
                        nc.tensor.matmul(pds[i + 1][:], M34s, uf,
                                         start=False, stop=False)
                    po = po_p.tile([128, BL], F32, tag="po",
                                   name=f"po{t}_{i}")
                    if i < 5:
                        nc.tensor.matmul(po[:], W4s, uf, start=True,
                                         stop=False)
                    pos[i] = po
                    if i == 5 and t + 1 < T:
                        for k in (0, 1):
                            pds_n[k] = pd_p.tile(
                                [128, BL], F32, tag="pd",
                                name=f"pd{t + 1}_{k}")
                            nc.tensor.matmul(pds_n[k][:], W2s,
                                             state_ap(k, t + 1),
                                             start=True, stop=False)
                    # DVE: copy u_g beside u_f (two-tensor DVE ops need
                    # equal base partitions), then the product.  Layer 5
                    # has no off-chain mg consumers, so fold the +1 into
                    # the copy and make po5 a single W4@mg2 matmul.
                    if i == 5:
                        nc.vector.tensor_scalar_add(us[0:64, BL:2 * BL],
                                                    us[64:128, 0:BL], 1.0)
                    else:
                        nc.vector.tensor_copy(us[0:64, BL:2 * BL],
                                              us[64:128, 0:BL])
                    mg = mg_p.tile([64, BL], BF16, tag="mg")
                    nc.vector.tensor_mul(mg[:], us[0:64, BL:2 * BL], uf)
                    # prod-gated chain group.
                    if i < 5:
                        nc.tensor.matmul(pds[i + 1][:], M34s, mg[:],
                                         start=False, stop=True)
                    nc.tensor.matmul(po[:], W4s, mg[:],
                                     start=(i == 5), stop=True)
                    # off-chain telescope pair AFTER the chain group: its
                    # ready half bypass-fills the last pre-chain gap.
                    if i + 2 <= 5:
                        nc.tensor.matmul(pds[i + 2][:], M34s, uf,
                                         start=False, stop=False)
                        nc.tensor.matmul(pds[i + 2][:], M34s, mg[:],
                                         start=False, stop=False)
                    # deferred residual add (j = i-1) off the chain.
                    # IN[4]/IN[5] are only read as dilated states d steps
                    # later; skip the add when that lands past T.
                    if i >= 1:
                        j = i - 1
                        if j <= 2 or t + DILATIONS[j] < T:
                            nc.vector.tensor_add(INS[j + 1][:, blk(t)],
                                                 pos[j][64:128, :],
                                                 INS[j][:, blk(t)])

                # -- tail
                relu_skip(4)
                nc.vector.tensor_scalar_max(sk[:, blk_s(5)],
                                            pos[5][0:64, :], 0.0)
                nc.tensor.matmul(ph[:], W5s[4], sk[:, blk_s(4)],
                                 start=False, stop=False)
                nc.tensor.matmul(ph[:], W5s[5], sk[:, blk_s(5)],
                                 start=False, stop=True)
                h = h_p.tile([HID, BL], BF16, tag="h")
                nc.vector.tensor_scalar_max(h[:], ph[:], 0.0)
                if t + 1 < T:
                    pin = pin_p.tile([64, BL], F32, tag="pin",
                                     name=f"pin{t + 1}")
                    nc.tensor.matmul(pin[:], W1s, featT[:, blk(t + 1)],
                                     start=True, stop=False)
                    nc.tensor.matmul(pin[:], As, h[:],
                                     start=False, stop=True)
                py = po_p.tile([1, BL], F32, tag="po", name=f"py{t}")
                nc.tensor.matmul(py[:], W6s, h[:], start=True, stop=True)
                nc.vector.tensor_scalar_add(yout_sb[0:1, blk(t)], py[:],
                                            b6f)
                if t % 6 == 5 or t == T - 2:
                    c0 = (t // 6) * 6 * BL
                    c1 = (t + 1) * BL
                    nc.sync.dma_start(y_out[0:1, c0:c1],
                                      yout_sb[0:1, c0:c1])
                if t + 1 < T:
                    for k in (2, 3):
                        pds_n[k] = pd_p.tile([128, BL], F32, tag="pd",
                                             name=f"pd{t + 1}_{k}")
                        nc.tensor.matmul(pds_n[k][:], W2s,
                                         state_ap(k, t + 1),
                                         start=True, stop=False)
                    pds = pds_n


    nc.compile()
    return nc


def _pack_inputs(decoder_features, decoder_init_input, encoder_states,
                 W1, W2, W3, W4, W5, W6):
    """Host-side shard + transpose + bf16-cast.  Returns per-core in_maps."""
    wbp = np.zeros((128, _CB), np.float32)
    wbp[0:64, _C_W2:_C_W2 + 64] = W2[:, 0:64]
    wbp[0:64, _C_W2 + 64:_C_W2 + 128] = 0.5 * W2[:, 64:128]
    wbp[0:64, _C_W3:_C_W3 + 64] = W3[:, 0:64]
    wbp[0:64, _C_W3 + 64:_C_W3 + 128] = 0.5 * W3[:, 64:128]
    M34 = 0.5 * (W4[:, 64:128] @ W3)                 # [64, 128]
    wbp[0:64, _C_M34:_C_M34 + 64] = M34[:, 0:64]
    wbp[0:64, _C_M34 + 64:_C_M34 + 128] = 0.5 * M34[:, 64:128]
    wbp[0:64, _C_W4:_C_W4 + 128] = 0.5 * W4
    for j in range(6):
        wbp[0:64, _C_W5 + j * 128:_C_W5 + (j + 1) * 128] = \
            W5[j * 64:(j + 1) * 64, :]
    wbp[0:16, _C_W1:_C_W1 + 64] = W1
    wbp[:, _C_A:_C_A + 64] = W6 @ W1[0:1, :]         # [128, 64]
    wbp[:, _C_W6:_C_W6 + 1] = W6
    wb_bits = _bf16(wbp)

    in_maps = []
    for c in range(N_CORES):
        s = slice(c * BL, (c + 1) * BL)
        # featT [16, T*BL]: row 0 blk 0 = init, zero elsewhere;
        # rows 1:16 = features^T.
        ft = np.zeros((16, T, BL), np.float32)
        ft[0, 0, :] = decoder_init_input[s, 0]
        ft[1:16] = decoder_features[s].transpose(2, 1, 0)
        # encpack [64, ENC_TOT*BL]
        ep = np.zeros((F, ENC_TOT, BL), np.float32)
        for i, d in enumerate(DILATIONS):
            n = ENC_N[i]
            ep[:, ENC_OFF[i]:ENC_OFF[i] + n, :] = \
                encoder_states[i, s, L - d:L - d + n, :].transpose(2, 1, 0)
        in_maps.append({
            "featsrc": _bf16(ft.reshape(16, T * BL)),
            "encpack": _bf16(ep.reshape(F, ENC_TOT * BL)),
            "wpackb": wb_bits,
        })
    return in_maps


def kernel(**inputs):
    decoder_features = np.asarray(inputs["decoder_features"], np.float32)
    decoder_init_input = np.asarray(inputs["decoder_init_input"], np.float32)
    encoder_states = np.asarray(inputs["encoder_states"], np.float32)
    W1 = np.asarray(inputs["W1"], np.float32)
    b1 = np.asarray(inputs["b1"], np.float32)
    W2 = np.asarray(inputs["W2"], np.float32)
    b2 = np.asarray(inputs["b2"], np.float32)
    W3 = np.asarray(inputs["W3"], np.float32)
    W4 = np.asarray(inputs["W4"], np.float32)
    b4 = np.asarray(inputs["b4"], np.float32)
    W5 = np.asarray(inputs["W5"], np.float32)
    b5 = np.asarray(inputs["b5"], np.float32)
    W6 = np.asarray(inputs["W6"], np.float32)
    b6 = np.asarray(inputs["b6"], np.float32)
    assert float(np.abs(b2).max()) == 0.0, "kernel assumes b2 == 0"
    assert float(np.abs(b4).max()) == 0.0, \
        "kernel's telescoped dilated accumulation assumes b4 == 0"
    assert float(np.abs(b5).max()) == 0.0, "kernel assumes b5 == 0"

    key = "nc"
    if key not in _CACHE:
        _CACHE[key] = _build(b1, b6)
    nc = _CACHE[key]

    in_maps = _pack_inputs(decoder_features, decoder_init_input,
                           encoder_states, W1, W2, W3, W4, W5, W6)
    res = run_bass_kernel_spmd(nc, in_maps, list(range(N_CORES)))

    out = np.empty((B, T, 1), np.float32)
    for c in range(N_CORES):
        y = res.results[c]["yout"].reshape(T, BL)
        out[c * BL:(c + 1) * BL, :, 0] = y.T
    return out
